# revision 1
# baseline (speedup 1.0000x reference)
"""CoPE multi-head attention Trainium2 kernel.

Sharding: 16 heads / 8 cores = 2 heads per core (head/tensor parallel).
Each core gets full q,k,v (pre-transposed on host) + its head-slice of the
projection weights, computes its 2 heads' attention + its partial output
projection; host sums the 8 partials and adds the output bias.

CoPE structure exploited: pos = reverse-cumsum of sigmoid gates clips at
npos-1=63.  For keys before a 256-wide tail suffix, pos >= 63 (verified at
runtime via a flag output), so cope == T[q,63], a per-row constant that
cancels in softmax.  Only the tail needs the real interpolated gather,
done via GPSIMD local_scatter (run-start positions -> table values) + a
sample-and-hold tensor_tensor_scan.
"""

import numpy as np

B, S, ND, NH, DH, NPOS = 1, 2048, 1024, 16, 64, 64
NCORES = 8
HPC = NH // NCORES          # heads per core = 2
DHC = HPC * DH              # head dims per core = 128
TAIL = 256                  # tail width (suffix of key axis)
KHEAD = S - TAIL            # 1792
NQB = S // 128              # 16 q blocks
NKB = S // 128              # 16 k blocks
NKB_HEAD = KHEAD // 128     # 14
GRP = 4                     # tail q-blocks per scatter group
TAILC = 192                 # columns of the tail that get the full CoPE chain
TAIL0 = TAIL - TAILC        # leading tail columns treated as clipped (delta=0)

_prog = None


def _build_program():
    import concourse.bacc as bacc
    import concourse.tile as tile
    from concourse import mybir

    dt = mybir.dt
    AF = mybir.ActivationFunctionType
    OP = mybir.AluOpType

    nc = bacc.Bacc("TRN2", target_bir_lowering=False, debug=False,
                   num_devices=NCORES)

    # ---- DRAM I/O ----
    qT_d = nc.dram_tensor("qT", [ND, S], dt.float32r, kind="ExternalInput").ap()
    kT_d = nc.dram_tensor("kT", [ND, S], dt.float32r, kind="ExternalInput").ap()
    vT_d = nc.dram_tensor("vT", [ND, S], dt.float32r, kind="ExternalInput").ap()
    wqT_d = nc.dram_tensor("wqT", [ND, DHC], dt.float32r, kind="ExternalInput").ap()
    wkT_d = nc.dram_tensor("wkT", [ND, DHC], dt.float32r, kind="ExternalInput").ap()
    wvT_d = nc.dram_tensor("wvT", [ND, DHC], dt.float32r, kind="ExternalInput").ap()
    woT_d = nc.dram_tensor("woT", [DHC, ND], dt.float32r, kind="ExternalInput").ap()
    bq_d = nc.dram_tensor("bq", [DHC, 1], dt.float32, kind="ExternalInput").ap()
    bk_d = nc.dram_tensor("bk", [DHC, 1], dt.float32, kind="ExternalInput").ap()  # pre-scaled by 1/8
    bv_d = nc.dram_tensor("bv", [DHC, 1], dt.float32, kind="ExternalInput").ap()
    pe_d = nc.dram_tensor("pe", [2 * DH, NPOS], dt.float32r, kind="ExternalInput").ap()
    iota_d = nc.dram_tensor("iota", [128, GRP * TAILC], dt.int16, kind="ExternalInput").ap()
    off_d = nc.dram_tensor("off", [128, GRP, TAILC], dt.float16, kind="ExternalInput").ap()
    ident_d = nc.dram_tensor("ident", [128, 128], dt.float32r, kind="ExternalInput").ap()
    out_d = nc.dram_tensor("out", [HPC, S, ND], dt.float32, kind="ExternalOutput").ap()
    flag_d = nc.dram_tensor("flag", [128, HPC * NQB], dt.float32, kind="ExternalOutput").ap()

    with tile.TileContext(nc) as tc:
        consts = tc.alloc_tile_pool(name="consts", bufs=1)
        big = tc.alloc_tile_pool(name="big", bufs=1)

        # ---- constants ----
        wqT = consts.tile([128, 8, DHC], dt.float32r, tag="wq")
        wkT = consts.tile([128, 8, DHC], dt.float32r, tag="wk")
        wvT = consts.tile([128, 8, DHC], dt.float32r, tag="wv")
        nc.sync.dma_start(out=wqT, in_=wqT_d.rearrange("(b p) d -> p b d", p=128))
        nc.sync.dma_start(out=wkT, in_=wkT_d.rearrange("(b p) d -> p b d", p=128))
        nc.sync.dma_start(out=wvT, in_=wvT_d.rearrange("(b p) d -> p b d", p=128))
        woT = [consts.tile([DH, ND], dt.float32r, tag=f"wo{h}", name=f"woT{h}") for h in range(HPC)]
        for h in range(HPC):
            nc.sync.dma_start(out=woT[h], in_=woT_d[h * DH:(h + 1) * DH, :])
        bq = consts.tile([DHC, 1], dt.float32, tag="bq")
        bk = consts.tile([DHC, 1], dt.float32, tag="bk")
        bv = consts.tile([DHC, 1], dt.float32, tag="bv")
        nc.sync.dma_start(out=bq, in_=bq_d)
        nc.sync.dma_start(out=bk, in_=bk_d)
        nc.sync.dma_start(out=bv, in_=bv_d)
        pe = consts.tile([2 * DH, NPOS], dt.float32r, tag="pe")
        nc.sync.dma_start(out=pe, in_=pe_d)
        iota = consts.tile([128, GRP * TAILC], dt.int16, tag="iota")
        nc.sync.dma_start(out=iota, in_=iota_d)
        offt = consts.tile([128, GRP, TAILC], dt.float16, tag="offt")
        nc.sync.dma_start(out=offt, in_=off_d)
        ident = consts.tile([128, 128], dt.float32r, tag="ident")
        nc.sync.dma_start(out=ident, in_=ident_d)
        rmask = consts.tile([128, GRP, TAILC], dt.float32, tag="rmask")
        nc.vector.memset(rmask, 1.0)
        nc.vector.memset(rmask[:, :, 0:1], 0.0)

        # persistent activations
        QT = big.tile([DHC, S], dt.float32r, tag="QT")   # [128 dh, 2048 s]
        KT = big.tile([DHC, S], dt.float32r, tag="KT")   # pre-scaled by 1/8
        Vn = big.tile([128, NKB, 131], dt.float32r, tag="Vn")  # per kblk: h0 V[0:64], ones 64, h1 V 66:130, ones 130
        flag_sb = big.tile([128, HPC * NQB], dt.float32, tag="flag")

        # ---- phase 1: projections ----
        with tc.tile_pool(name="xstage", bufs=3) as xstage, \
             tc.tile_pool(name="proj_ps", bufs=1, space="PSUM") as proj_ps, \
             tc.tile_pool(name="vt_ps", bufs=2, space="PSUM") as vt_ps, \
             tc.tile_pool(name="vstage", bufs=1) as vstage:
            VT = vstage.tile([DHC, S], dt.float32r, tag="VT")
            for name, x_d, wT, bias, dest, scale in (
                ("q", qT_d, wqT, bq, QT, 1.0),
                ("k", kT_d, wkT, bk, KT, 0.125),
                ("v", vT_d, wvT, bv, VT, 1.0),
            ):
                ps_chunks = [proj_ps.tile([DHC, 512], dt.float32, tag=f"pp{c}", name=f"pp_{name}_{c}") for c in range(4)]
                for db in range(8):
                    xt = xstage.tile([128, S], dt.float32r, tag="xt")
                    nc.sync.dma_start(out=xt, in_=x_d[db * 128:(db + 1) * 128, :])
                    for c in range(4):
                        nc.tensor.matmul(out=ps_chunks[c], lhsT=wT[:, db, :],
                                         rhs=xt[:, c * 512:(c + 1) * 512],
                                         start=(db == 0), stop=(db == 7))
                for c in range(4):
                    nc.scalar.activation(out=dest[:, c * 512:(c + 1) * 512], in_=ps_chunks[c],
                                         func=AF.Identity, bias=bias, scale=scale)
            # V: transpose [dh,s] -> [s,dh] per 128-block, pack into Vn with ones cols
            nc.vector.memset(Vn[:, :, 64:65].bitcast(dt.float32), 1.0)
            nc.vector.memset(Vn[:, :, 130:131].bitcast(dt.float32), 1.0)
            for kb in range(NKB):
                pt = vt_ps.tile([128, 128], dt.float32r, tag="vt")
                nc.tensor.transpose(pt, VT[:, kb * 128:(kb + 1) * 128], ident)
                dst = bass_ap_2range(Vn, kb)
                nc.scalar.activation(out=dst, in_=pt.rearrange("p (b d) -> p b d", b=2),
                                     func=AF.Identity)

        # ---- phase 2: cope tables T'[q,n], dT'[q,n] (fp16) per head ----
        Tp = [big.tile([128, NQB, NPOS], dt.float16, tag=f"Tp{h}", name=f"Tp{h}") for h in range(HPC)]
        dTp = [big.tile([128, NQB, NPOS], dt.float16, tag=f"dTp{h}", name=f"dTp{h}") for h in range(HPC)]
        with tc.tile_pool(name="tt_ps", bufs=2, space="PSUM") as tt_ps, \
             tc.tile_pool(name="tt_sb", bufs=2) as tt_sb:
            for h in range(HPC):
                for gg in range(2):
                    ps = tt_ps.tile([128, 8, NPOS], dt.float32, tag="ttp")
                    for qq in range(8):
                        qb = gg * 8 + qq
                        nc.tensor.matmul(out=ps[:, qq, :],
                                         lhsT=QT[h * DH:(h + 1) * DH, qb * 128:(qb + 1) * 128],
                                         rhs=pe[h * DH:(h + 1) * DH, :], start=True, stop=True)
                    tsb = tt_sb.tile([128, 8, NPOS], dt.float32, tag="tsb")
                    nc.scalar.copy(out=tsb, in_=ps)
                    for qq in range(8):
                        qb = gg * 8 + qq
                        nc.vector.tensor_scalar(out=Tp[h][:, qb, :], in0=tsb[:, qq, :],
                                                scalar1=tsb[:, qq, 63:64], scalar2=None,
                                                op0=OP.subtract)
                    nc.vector.tensor_tensor(out=dTp[h][:, gg * 8:(gg + 1) * 8, 0:63],
                                            in0=tsb[:, :, 1:64], in1=tsb[:, :, 0:63],
                                            op=OP.subtract)
                    nc.vector.memset(dTp[h][:, gg * 8:(gg + 1) * 8, 63:64], 0.0)

        # ---- phases 3-5: tail cope -> exps -> PV -> Wo, sequenced so head1's
        # tail overlaps head0's k-loop and head0's Wo overlaps head1's k-loop ----
        hoT = [big.tile([65, S], dt.float32r, tag=f"hoT{h}", name=f"hoT{h}") for h in range(HPC)]
        rden = [big.tile([128, NQB], dt.float32, tag=f"rden{h}", name=f"rden{h}") for h in range(HPC)]
        with tc.tile_pool(name="tmp_pool", bufs=1) as tmp_pool, \
             tc.tile_pool(name="tl", bufs=2) as tl, \
             tc.tile_pool(name="tls", bufs=4) as tls, \
             tc.tile_pool(name="sct_ps", bufs=1, space="PSUM") as sct_ps, \
             tc.tile_pool(name="s_ps", bufs=2, space="PSUM") as s_ps, \
             tc.tile_pool(name="pv_ps", bufs=1, space="PSUM") as pv_ps, \
             tc.tile_pool(name="et", bufs=2) as et, \
             tc.tile_pool(name="sc_pool", bufs=2) as sc_pool, \
             tc.tile_pool(name="sct_pool", bufs=1) as sct_pool:
            wo_pools = {}
            tail_tmps = [tmp_pool.tile([128, NQB * TAIL], dt.float32, tag=f"tt{h}",
                                       name=f"tail_tmp{h}") for h in range(HPC)]
            scts = [None, None]

            def tail_chain(h):
                hq = QT[h * DH:(h + 1) * DH, :]
                hk = KT[h * DH:(h + 1) * DH, :]
                tail_tmp = tail_tmps[h]
                # -- tail chain --
                for g in range(NQB // GRP):
                    Ssb = tl.tile([128, GRP, TAIL], dt.float32, tag="Ssb")
                    for jj in range(GRP // 2):
                        stp = st_ps.tile([128, 2, TAIL], dt.float32, tag="stp")
                        for t in range(2):
                            qb = g * GRP + 2 * jj + t
                            nc.tensor.matmul(out=stp[:, t, :],
                                             lhsT=hq[:, qb * 128:(qb + 1) * 128],
                                             rhs=hk[:, KHEAD:S], start=True, stop=True)
                        nc.vector.tensor_copy(out=Ssb[:, 2 * jj:2 * jj + 2, :], in_=stp)
                    spsc = Ssb[:, :, TAIL0:TAIL]
                    gts = tl.tile([128, GRP, TAILC], dt.float32, tag="gts")
                    cfb = tl.tile([128, GRP, TAILC], dt.float16, tag="cfb")
                    mifb = tl.tile([128, GRP, TAILC], dt.float16, tag="mifb")
                    posb = tl.tile([128, GRP, TAILC], dt.float32, tag="posb")
                    wb = tl.tile([128, GRP, TAILC], dt.float32, tag="wb")
                    eqb = tl.tile([128, GRP, TAILC], dt.float16, tag="eqb")
                    mi16 = tl.tile([128, GRP, TAILC], dt.int16, tag="mi16")
                    scA = tl.tile([128, GRP, TAILC], dt.float16, tag="scA")
                    scB = tl.tile([128, GRP, TAILC], dt.float16, tag="scB")
                    c16 = tl.tile([128, GRP, TAILC], dt.int16, tag="c16")
                    gsb = tls.tile([128, GRP], dt.float32, tag="gsb")
                    for j in range(GRP):
                        nc.scalar.activation(out=gts[:, j, :], in_=spsc[:, j, :], func=AF.Sigmoid,
                                             accum_out=gsb[:, j:j + 1])
                    nc.vector.tensor_copy(out=flag_sb[:, h * NQB + g * GRP:h * NQB + (g + 1) * GRP],
                                          in_=gsb)
                    # col0 of each sub-block: g[0] - gsum (seeds the chained scan)
                    nc.vector.tensor_tensor(out=gts[:, :, 0:1], in0=gts[:, :, 0:1],
                                            in1=gsb.rearrange("p (g o) -> p g o", o=1),
                                            op=OP.subtract)
                    # chained cumsum with reset: state = rmask*state + g'
                    nc.vector.tensor_tensor_scan(
                        out=posb.rearrange("p g t -> p (g t)"),
                        data0=rmask.rearrange("p g t -> p (g t)"),
                        data1=gts.rearrange("p g t -> p (g t)"), initial=0.0,
                        op0=OP.mult, op1=OP.add)
                    # posb now holds cs - gsum (incl. own g); rev-cumsum = g - (cs - gsum)
                    # ... but col0 of gts was modified; restore contribution via using
                    # original gate value: g'[0] = g[0]-gsum, cs'[*] = cs - gsum, and
                    # pos = g - cs' works with the *original* g, so rebuild col0 first
                    nc.vector.tensor_tensor(out=gts[:, :, 0:1], in0=gts[:, :, 0:1],
                                            in1=gsb.rearrange("p (g o) -> p g o", o=1),
                                            op=OP.add)
                    nc.vector.scalar_tensor_tensor(out=posb, in0=posb, scalar=-1.0,
                                                   in1=gts, op0=OP.mult, op1=OP.add)
                    nc.vector.tensor_scalar(out=c16, in0=posb, scalar1=0.5, scalar2=62.0,
                                            op0=OP.subtract, op1=OP.min)
                    nc.vector.tensor_copy(out=cfb, in_=c16)
                    nc.vector.scalar_tensor_tensor(out=wb, in0=posb, scalar=63.0,
                                                   in1=cfb, op0=OP.min, op1=OP.subtract)
                    nc.vector.tensor_tensor(out=eqb[:, :, 1:], in0=cfb[:, :, 1:],
                                            in1=cfb[:, :, :-1], op=OP.not_equal)
                    nc.vector.memset(eqb[:, :, 0:1], 1.0)
                    nc.vector.tensor_add(out=mifb, in0=cfb, in1=offt)
                    nc.vector.scalar_tensor_tensor(out=mifb, in0=mifb, scalar=1.0,
                                                   in1=eqb, op0=OP.add, op1=OP.mult)
                    nc.vector.tensor_scalar(out=mi16, in0=mifb, scalar1=1.0, scalar2=None,
                                            op0=OP.subtract)
                    nc.vector.tensor_scalar(out=eqb, in0=eqb, scalar1=-1.0, scalar2=-1.0,
                                            op0=OP.add, op1=OP.mult)
                    ptab = tl.tile([128, GRP * NPOS], dt.int16, tag="ptab")
                    nc.gpsimd.local_scatter(out_ap=ptab[:], data_ap=iota[:], idxs_ap=mi16[:],
                                            channels=128, num_elems=GRP * NPOS,
                                            num_idxs=GRP * TAILC)
                    nc.vector.tensor_scalar(out=ptab, in0=ptab, scalar1=1, scalar2=None,
                                            op0=OP.subtract)
                    nc.gpsimd.local_scatter(out_ap=scA[:], data_ap=Tp[h][:, g * GRP:(g + 1) * GRP, :],
                                            idxs_ap=ptab[:], channels=128,
                                            num_elems=GRP * TAILC, num_idxs=GRP * NPOS)
                    nc.gpsimd.local_scatter(out_ap=scB[:], data_ap=dTp[h][:, g * GRP:(g + 1) * GRP, :],
                                            idxs_ap=ptab[:], channels=128,
                                            num_elems=GRP * TAILC, num_idxs=GRP * NPOS)
                    Ab = tl.tile([128, GRP, TAILC], dt.float32, tag="Ab")
                    Bb = tl.tile([128, GRP, TAILC], dt.float32, tag="Bb")
                    # col0 of every sub-block is a forced run-start, so one chained
                    # scan over the flattened group self-resets at block boundaries
                    nc.vector.tensor_tensor_scan(
                        out=Ab.rearrange("p g t -> p (g t)"),
                        data0=eqb.rearrange("p g t -> p (g t)"),
                        data1=scA.rearrange("p g t -> p (g t)"), initial=0.0,
                        op0=OP.mult, op1=OP.add)
                    nc.vector.tensor_tensor_scan(
                        out=Bb.rearrange("p g t -> p (g t)"),
                        data0=eqb.rearrange("p g t -> p (g t)"),
                        data1=scB.rearrange("p g t -> p (g t)"), initial=0.0,
                        op0=OP.mult, op1=OP.add)
                    nc.vector.tensor_mul(out=Bb, in0=wb, in1=Bb)
                    nc.vector.tensor_add(out=Ab, in0=Ab, in1=Bb)
                    tt_dst = tail_tmp.rearrange("p (q t) -> p q t", t=TAIL)[:, g * GRP:(g + 1) * GRP, :]
                    nc.vector.tensor_add(out=tt_dst[:, :, TAIL0:TAIL], in0=Ab, in1=spsc)
                    nc.scalar.activation(out=tt_dst[:, :, 0:TAIL0], in_=Ssb[:, :, 0:TAIL0],
                                         func=AF.Copy)
            def sct_transpose(h):
                sct = sct_pool.tile([128, 2, S], dt.float32r, tag="sct", name=f"sct{h}")
                scts[h] = sct
                tail_tmp = tail_tmps[h]
                # -- tail exp + transpose --
                for qb in range(NQB):
                    etile = et.tile([128, TAIL], dt.float32r, tag="etile")
                    nc.scalar.activation(out=etile, in_=tail_tmp[:, qb * TAIL:(qb + 1) * TAIL],
                                         func=AF.Exp)
                    pt = sct_ps.tile([128, 2, 128], dt.float32r, tag="sctp")
                    for t in range(2):
                        nc.tensor.transpose(pt[:, t, :], etile[:, t * 128:(t + 1) * 128], ident)
                    nc.vector.tensor_copy(out=sct.rearrange("p t (q c) -> p t q c", c=128)[:, :, qb, :],
                                          in_=pt)
            def kloop(h):
                hq = QT[h * DH:(h + 1) * DH, :]
                hk = KT[h * DH:(h + 1) * DH, :]
                sct = scts[h]
                # -- k loop: exp + PV --
                pv = pv_ps.tile([65, S], dt.float32, tag="pv")
                for kb in range(NKB):
                    if kb < NKB_HEAD:
                        sc = sc_pool.tile([128, S], dt.float32r, tag="sc")
                        for c in range(4):
                            sp = s_ps.tile([128, 512], dt.float32, tag="sp")
                            nc.tensor.matmul(out=sp,
                                             lhsT=hk[:, kb * 128:(kb + 1) * 128],
                                             rhs=hq[:, c * 512:(c + 1) * 512],
                                             start=True, stop=True)
                            nc.scalar.activation(out=sc[:, c * 512:(c + 1) * 512], in_=sp,
                                                 func=AF.Exp)
                    else:
                        sc = sct[:, kb - NKB_HEAD, :]
                    lhs = Vn[:, kb, 0:65] if h == 0 else Vn[:, kb, 66:131]
                    for c in range(4):
                        nc.tensor.matmul(out=pv[:, c * 512:(c + 1) * 512], lhsT=lhs,
                                         rhs=sc[:, c * 512:(c + 1) * 512],
                                         start=(kb == 0), stop=(kb == NKB - 1))
                nc.scalar.copy(out=hoT[h], in_=pv)
            def dn(h):
                # denominators for this head (shares the sct_ps slot)
                dcol = tls.tile([128, NQB], dt.float32, tag=f"dcol{h}", name=f"dcol{h}")
                for qb in range(NQB):
                    ptd = sct_ps.tile([128, 2, 128], dt.float32r, tag="sctp", name=f"dnp{h}_{qb}")
                    nc.tensor.transpose(ptd[:, 0, 0:65].bitcast(dt.float32),
                                        hoT[h][:, qb * 128:(qb + 1) * 128].bitcast(dt.float32),
                                        ident[0:65, 0:65].bitcast(dt.float32))
                    nc.scalar.copy(out=dcol[:, qb:qb + 1], in_=ptd[:, 0, 64:65].bitcast(dt.float32))
                nc.vector.reciprocal(out=rden[h], in_=dcol)
            def wo(h, evac_dve):
                for sb in range(NQB):
                    for c in range(2):
                        wop = wo_pools['ps'].tile([128, 512], dt.float32, tag="wop",
                                         name=f"wop{h}_{sb}_{c}")
                        nc.tensor.matmul(out=wop,
                                         lhsT=hoT[h][0:64, sb * 128:(sb + 1) * 128],
                                         rhs=woT[h][:, c * 512:(c + 1) * 512],
                                         start=True, stop=True)
                        ob = wo_pools['sb'].tile([128, 512], dt.float32, tag="ob",
                                        name=f"ob{h}_{sb}_{c}")
                        if evac_dve or c == 1:
                            nc.vector.tensor_scalar(out=ob, in0=wop,
                                                    scalar1=rden[h][:, sb:sb + 1],
                                                    scalar2=None, op0=OP.mult)
                        else:
                            nc.scalar.activation(out=ob, in_=wop, func=AF.Identity,
                                                 scale=rden[h][:, sb:sb + 1])
                        nc.sync.dma_start(
                            out=out_d[h, sb * 128:(sb + 1) * 128, c * 512:(c + 1) * 512],
                            in_=ob)

            with tc.tile_pool(name="st_ps", bufs=1, space="PSUM") as st_ps:
                tail_chain(0)
                sct_transpose(0)
                kloop(0)
                dn(0)
                tail_chain(1)
            with tc.tile_pool(name="wo_ps", bufs=1, space="PSUM") as _wops, \
                 tc.tile_pool(name="wo_sb", bufs=2) as _wosb:
                wo_pools['ps'], wo_pools['sb'] = _wops, _wosb
                sct_transpose(1)
                wo(0, evac_dve=True)
                kloop(1)
                dn(1)
                wo(1, evac_dve=False)

        nc.sync.dma_start(out=flag_d, in_=flag_sb)
        big.release()
        consts.release()

    nc.compile()
    return nc


def bass_ap_2range(Vn, kb):
    """out AP [128, 2, 64] covering Vn[:, kb, 0:64] and Vn[:, kb, 66:130]."""
    import concourse.bass as bass
    base = Vn[:, kb, 0:64]
    ap = [list(base.ap[0]), [66, 2], [1, 64]]
    return bass.AP(base.tensor, base.offset, [list(p) for p in ap])


def _prepare_maps(q, k, v, Wq_w, Wq_b, Wk_w, Wk_b, Wv_w, Wv_b, Wo_w, Wo_b, pos_emb):
    f32 = np.float32
    qT = np.ascontiguousarray(q[0].T.astype(f32))
    kT = np.ascontiguousarray(k[0].T.astype(f32))
    vT = np.ascontiguousarray(v[0].T.astype(f32))
    iota = np.tile(np.arange(1, GRP * TAILC + 1, dtype=np.int16), (128, 1))
    off = np.tile(np.repeat(np.arange(GRP, dtype=np.float16) * NPOS, TAILC), (128, 1)).reshape(128, GRP, TAILC)
    ident = np.eye(128, dtype=f32)
    pe = np.ascontiguousarray(pos_emb.astype(f32))
    in_maps = []
    for c in range(NCORES):
        r0 = c * DHC
        sl = slice(r0, r0 + DHC)
        in_maps.append({
            "qT": qT, "kT": kT, "vT": vT,
            "wqT": np.ascontiguousarray(Wq_w[sl, :].T.astype(f32)),
            "wkT": np.ascontiguousarray(Wk_w[sl, :].T.astype(f32)),
            "wvT": np.ascontiguousarray(Wv_w[sl, :].T.astype(f32)),
            "woT": np.ascontiguousarray(Wo_w[:, sl].T.astype(f32)),
            "bq": np.ascontiguousarray(Wq_b[sl].astype(f32)[:, None]),
            "bk": np.ascontiguousarray((Wk_b[sl] * 0.125).astype(f32)[:, None]),
            "bv": np.ascontiguousarray(Wv_b[sl].astype(f32)[:, None]),
            "pe": np.concatenate([pe, pe], axis=0), "iota": iota, "ident": ident, "off": np.ascontiguousarray(off),
        })
    return in_maps


def _reference_fallback(q, k, v, Wq_w, Wq_b, Wk_w, Wk_b, Wv_w, Wv_b, Wo_w, Wo_b,
                        pos_emb, nheads):
    """Exact numpy fallback (used only if the clip-safety flag fails)."""
    b, s, ndims = q.shape
    d = ndims // nheads

    def heads(x, W, bb):
        y = x.reshape(-1, ndims) @ W.T + bb
        return y.reshape(b, s, nheads, d).transpose(0, 2, 1, 3)

    Q = heads(q, Wq_w, Wq_b)
    K = heads(k, Wk_w, Wk_b)
    V = heads(v, Wv_w, Wv_b)
    logits = np.einsum("bhqd,bhkd->bhqk", Q, K) / np.sqrt(d)
    npos = pos_emb.shape[-1]
    gates = 1.0 / (1.0 + np.exp(-logits))
    pos = np.flip(np.cumsum(np.flip(gates, -1), -1), -1)
    pos = np.minimum(pos, npos - 1)
    pc = np.ceil(pos).astype(np.int64)
    pf = np.floor(pos).astype(np.int64)
    li = np.einsum("bhqd,dn->bhqn", Q, pos_emb)
    lc = np.take_along_axis(li, pc, -1)
    lf = np.take_along_axis(li, pf, -1)
    w = pos - pf
    cope = lc * w + lf * (1.0 - w)
    x = logits + cope
    x = x - x.max(-1, keepdims=True)
    e = np.exp(x)
    scores = e / e.sum(-1, keepdims=True)
    out = np.einsum("bhqk,bhkd->bhqd", scores, V)
    out = out.transpose(0, 2, 1, 3).reshape(b, s, ndims)
    return (out @ Wo_w.T + Wo_b).astype(np.float32)


def kernel(q, k, v, Wq_w, Wq_b, Wk_w, Wk_b, Wv_w, Wv_b, Wo_w, Wo_b, pos_emb,
           nheads, _want_trace=False):
    global _prog
    from concourse.bass_utils import run_bass_kernel_spmd

    q = np.asarray(q); k = np.asarray(k); v = np.asarray(v)
    Wq_w = np.asarray(Wq_w); Wq_b = np.asarray(Wq_b)
    Wk_w = np.asarray(Wk_w); Wk_b = np.asarray(Wk_b)
    Wv_w = np.asarray(Wv_w); Wv_b = np.asarray(Wv_b)
    Wo_w = np.asarray(Wo_w); Wo_b = np.asarray(Wo_b)
    pos_emb = np.asarray(pos_emb)

    if _prog is None:
        _prog = _build_program()
    in_maps = _prepare_maps(q, k, v, Wq_w, Wq_b, Wk_w, Wk_b, Wv_w, Wv_b,
                            Wo_w, Wo_b, pos_emb)
    res = run_bass_kernel_spmd(_prog, in_maps, core_ids=list(range(NCORES)),
                               trace=_want_trace)
    flag_min = min(float(r["flag"].min()) for r in res.results)
    if flag_min < float(NPOS - 1):
        out = _reference_fallback(q, k, v, Wq_w, Wq_b, Wk_w, Wk_b, Wv_w, Wv_b,
                                  Wo_w, Wo_b, pos_emb, int(nheads))
        return out if not _want_trace else (out, res)
    total = res.results[0]["out"].astype(np.float64).sum(axis=0)
    for r in res.results[1:]:
        total = total + r["out"].astype(np.float64).sum(axis=0)
    out = (total + Wo_b.astype(np.float64)).astype(np.float32)[None]
    return out if not _want_trace else (out, res)



# revision 2
# speedup vs baseline: 1.1968x; 1.1968x over previous
"""CoPE multi-head attention Trainium2 kernel.

Sharding: 16 heads / 8 cores = 2 heads per core (head/tensor parallel).
Each core gets full q,k,v (pre-transposed on host) + its head-slice of the
projection weights, computes its 2 heads' attention + its partial output
projection; host sums the 8 partials and adds the output bias.

CoPE structure exploited: pos = reverse-cumsum of sigmoid gates clips at
npos-1=63.  For keys before a 256-wide tail suffix, pos >= 63 (verified at
runtime via a flag output), so cope == T[q,63], a per-row constant that
cancels in softmax.  Only the tail needs the real interpolated gather,
done via GPSIMD local_scatter (run-start positions -> table values) + a
sample-and-hold tensor_tensor_scan.

Perf notes: all matmul operands are bf16 (FWL weight loads, half DMA);
gates use sigmoid(x) = 0.5*tanh(x/2)+0.5 so the Activation engine stays
on the exp_and_others table set (no ACT_TABLE_LOAD churn); outputs are
fp16 partials summed on host.
"""

import numpy as np

B, S, ND, NH, DH, NPOS = 1, 2048, 1024, 16, 64, 64
NCORES = 8
HPC = NH // NCORES          # heads per core = 2
DHC = HPC * DH              # head dims per core = 128
TAIL = 256                  # tail width (suffix of key axis)
KHEAD = S - TAIL            # 1792
NQB = S // 128              # 16 q blocks
NKB = S // 128              # 16 k blocks
NKB_HEAD = KHEAD // 128     # 14
GRP = 4                     # tail q-blocks per scatter group
TAILC = 192                 # columns of the tail that get the full CoPE chain
TAIL0 = TAIL - TAILC        # leading tail columns treated as clipped (delta=0)

_prog = None


def _build_program():
    import concourse.bacc as bacc
    import concourse.tile as tile
    from concourse import mybir

    dt = mybir.dt
    AF = mybir.ActivationFunctionType
    OP = mybir.AluOpType

    nc = bacc.Bacc("TRN2", target_bir_lowering=False, debug=False,
                   num_devices=NCORES)

    # ---- DRAM I/O ----
    qT_d = nc.dram_tensor("qT", [ND, S], dt.bfloat16, kind="ExternalInput").ap()
    kT_d = nc.dram_tensor("kT", [ND, S], dt.bfloat16, kind="ExternalInput").ap()
    vT_d = nc.dram_tensor("vT", [ND, S], dt.bfloat16, kind="ExternalInput").ap()
    wqT_d = nc.dram_tensor("wqT", [ND, DHC], dt.bfloat16, kind="ExternalInput").ap()
    wkT_d = nc.dram_tensor("wkT", [ND, DHC], dt.bfloat16, kind="ExternalInput").ap()
    wvT_d = nc.dram_tensor("wvT", [ND, DHC], dt.bfloat16, kind="ExternalInput").ap()
    woT_d = nc.dram_tensor("woT", [DHC, ND], dt.float32r, kind="ExternalInput").ap()
    bq_d = nc.dram_tensor("bq", [DHC, 1], dt.float32, kind="ExternalInput").ap()
    bk_d = nc.dram_tensor("bk", [DHC, 1], dt.float32, kind="ExternalInput").ap()  # pre-scaled by 1/8
    bv_d = nc.dram_tensor("bv", [DHC, 1], dt.float32, kind="ExternalInput").ap()
    pe_d = nc.dram_tensor("pe", [2 * DH, NPOS], dt.bfloat16, kind="ExternalInput").ap()
    iota_d = nc.dram_tensor("iota", [128, GRP * TAILC], dt.int16, kind="ExternalInput").ap()
    off_d = nc.dram_tensor("off", [128, GRP, TAILC], dt.float16, kind="ExternalInput").ap()
    identb_d = nc.dram_tensor("identb", [128, 128], dt.bfloat16, kind="ExternalInput").ap()
    ident32_d = nc.dram_tensor("ident32", [65, 65], dt.float32, kind="ExternalInput").ap()
    out_d = nc.dram_tensor("out", [HPC, S, ND], dt.float16, kind="ExternalOutput").ap()
    flag_d = nc.dram_tensor("flag", [128, HPC * NQB], dt.float32, kind="ExternalOutput").ap()

    with tile.TileContext(nc) as tc:
        consts = tc.alloc_tile_pool(name="consts", bufs=1)
        big = tc.alloc_tile_pool(name="big", bufs=1)

        # ---- constants ----
        wqT = consts.tile([128, 8, DHC], dt.bfloat16, tag="wq")
        wkT = consts.tile([128, 8, DHC], dt.bfloat16, tag="wk")
        wvT = consts.tile([128, 8, DHC], dt.bfloat16, tag="wv")
        nc.sync.dma_start(out=wqT, in_=wqT_d.rearrange("(b p) d -> p b d", p=128))
        nc.sync.dma_start(out=wkT, in_=wkT_d.rearrange("(b p) d -> p b d", p=128))
        nc.sync.dma_start(out=wvT, in_=wvT_d.rearrange("(b p) d -> p b d", p=128))
        woT = [consts.tile([DH, ND], dt.float32r, tag=f"wo{h}", name=f"woT{h}") for h in range(HPC)]
        for h in range(HPC):
            nc.sync.dma_start(out=woT[h], in_=woT_d[h * DH:(h + 1) * DH, :])
        bq = consts.tile([DHC, 1], dt.float32, tag="bq")
        bk = consts.tile([DHC, 1], dt.float32, tag="bk")
        bv = consts.tile([DHC, 1], dt.float32, tag="bv")
        nc.sync.dma_start(out=bq, in_=bq_d)
        nc.sync.dma_start(out=bk, in_=bk_d)
        nc.sync.dma_start(out=bv, in_=bv_d)
        pe = consts.tile([2 * DH, NPOS], dt.bfloat16, tag="pe")
        nc.sync.dma_start(out=pe, in_=pe_d)
        iota = consts.tile([128, GRP * TAILC], dt.int16, tag="iota")
        nc.sync.dma_start(out=iota, in_=iota_d)
        offt = consts.tile([128, GRP, TAILC], dt.float16, tag="offt")
        nc.sync.dma_start(out=offt, in_=off_d)
        identb = consts.tile([128, 128], dt.bfloat16, tag="identb")
        nc.sync.dma_start(out=identb, in_=identb_d)
        ident32 = consts.tile([65, 65], dt.float32, tag="ident32")
        nc.sync.dma_start(out=ident32, in_=ident32_d)
        rmask = consts.tile([128, GRP, TAILC], dt.float32, tag="rmask")
        nc.vector.memset(rmask, 1.0)
        nc.vector.memset(rmask[:, :, 0:1], 0.0)

        # persistent activations
        QT = big.tile([DHC, S], dt.bfloat16, tag="QT")   # [128 dh, 2048 s]
        KT = big.tile([DHC, S], dt.bfloat16, tag="KT")   # pre-scaled by 1/8
        Vn = big.tile([128, NKB, 131], dt.bfloat16, tag="Vn")  # per kblk: h0 V[0:64], ones 64, h1 V 66:130, ones 130
        flag_sb = big.tile([128, HPC * NQB], dt.float32, tag="flag")

        # ---- phase 1: projections ----
        with tc.tile_pool(name="xstage", bufs=3) as xstage, \
             tc.tile_pool(name="proj_ps", bufs=1, space="PSUM") as proj_ps, \
             tc.tile_pool(name="vt_ps", bufs=2, space="PSUM") as vt_ps, \
             tc.tile_pool(name="vstage", bufs=1) as vstage:
            VT = vstage.tile([DHC, S], dt.bfloat16, tag="VT")
            for name, x_d, wT, bias, dest, scale in (
                ("q", qT_d, wqT, bq, QT, 1.0),
                ("k", kT_d, wkT, bk, KT, 0.125),
                ("v", vT_d, wvT, bv, VT, 1.0),
            ):
                ps_chunks = [proj_ps.tile([DHC, 512], dt.float32, tag=f"pp{c}", name=f"pp_{name}_{c}") for c in range(4)]
                for db in range(8):
                    xt = xstage.tile([128, S], dt.bfloat16, tag="xt")
                    nc.sync.dma_start(out=xt, in_=x_d[db * 128:(db + 1) * 128, :])
                    for c in range(4):
                        nc.tensor.matmul(out=ps_chunks[c], lhsT=wT[:, db, :],
                                         rhs=xt[:, c * 512:(c + 1) * 512],
                                         start=(db == 0), stop=(db == 7))
                for c in range(4):
                    nc.scalar.activation(out=dest[:, c * 512:(c + 1) * 512], in_=ps_chunks[c],
                                         func=AF.Identity, bias=bias, scale=scale)
            # V: transpose [dh,s] -> [s,dh] per 128-block, pack into Vn with ones cols
            nc.vector.memset(Vn[:, :, 64:65], 1.0)
            nc.vector.memset(Vn[:, :, 130:131], 1.0)
            for kb in range(NKB):
                pt = vt_ps.tile([128, 128], dt.bfloat16, tag="vt")
                nc.tensor.transpose(pt, VT[:, kb * 128:(kb + 1) * 128], identb)
                dst = bass_ap_2range(Vn, kb)
                nc.scalar.activation(out=dst, in_=pt.rearrange("p (b d) -> p b d", b=2),
                                     func=AF.Identity)

        # ---- phase 2: cope tables T'[q,n], dT'[q,n] (fp16) per head ----
        Tp = [big.tile([128, NQB, NPOS], dt.float16, tag=f"Tp{h}", name=f"Tp{h}") for h in range(HPC)]
        dTp = [big.tile([128, NQB, NPOS], dt.float16, tag=f"dTp{h}", name=f"dTp{h}") for h in range(HPC)]
        with tc.tile_pool(name="tt_ps", bufs=2, space="PSUM") as tt_ps, \
             tc.tile_pool(name="tt_sb", bufs=2) as tt_sb:
            for h in range(HPC):
                for gg in range(2):
                    ps = tt_ps.tile([128, 8, NPOS], dt.float32, tag="ttp")
                    for qq in range(8):
                        qb = gg * 8 + qq
                        nc.tensor.matmul(out=ps[:, qq, :],
                                         lhsT=QT[h * DH:(h + 1) * DH, qb * 128:(qb + 1) * 128],
                                         rhs=pe[h * DH:(h + 1) * DH, :], start=True, stop=True)
                    tsb = tt_sb.tile([128, 8, NPOS], dt.float32, tag="tsb")
                    nc.scalar.copy(out=tsb, in_=ps)
                    for qq in range(8):
                        qb = gg * 8 + qq
                        nc.vector.tensor_scalar(out=Tp[h][:, qb, :], in0=tsb[:, qq, :],
                                                scalar1=tsb[:, qq, 63:64], scalar2=None,
                                                op0=OP.subtract)
                    nc.vector.tensor_tensor(out=dTp[h][:, gg * 8:(gg + 1) * 8, 0:63],
                                            in0=tsb[:, :, 1:64], in1=tsb[:, :, 0:63],
                                            op=OP.subtract)
                    nc.vector.memset(dTp[h][:, gg * 8:(gg + 1) * 8, 63:64], 0.0)

        # ---- phases 3-5: tail cope -> exps -> PV -> Wo, sequenced so head1's
        # tail overlaps head0's k-loop and head0's Wo overlaps head1's k-loop ----
        hoT = [big.tile([65, S], dt.float32r, tag=f"hoT{h}", name=f"hoT{h}") for h in range(HPC)]
        rden = [big.tile([128, NQB], dt.float32, tag=f"rden{h}", name=f"rden{h}") for h in range(HPC)]
        with tc.tile_pool(name="tmp_pool", bufs=1) as tmp_pool, \
             tc.tile_pool(name="tl", bufs=2) as tl, \
             tc.tile_pool(name="tls", bufs=4) as tls, \
             tc.tile_pool(name="sct_ps", bufs=1, space="PSUM") as sct_ps, \
             tc.tile_pool(name="s_ps", bufs=2, space="PSUM") as s_ps, \
             tc.tile_pool(name="pv_ps", bufs=1, space="PSUM") as pv_ps, \
             tc.tile_pool(name="et", bufs=2) as et, \
             tc.tile_pool(name="sc_pool", bufs=2) as sc_pool, \
             tc.tile_pool(name="sct_pool", bufs=1) as sct_pool:
            wo_pools = {}
            tail_tmps = [tmp_pool.tile([128, NQB * TAIL], dt.float16, tag=f"tt{h}",
                                       name=f"tail_tmp{h}") for h in range(HPC)]
            scts = [None, None]

            def tail_chain(h):
                hq = QT[h * DH:(h + 1) * DH, :]
                hk = KT[h * DH:(h + 1) * DH, :]
                tail_tmp = tail_tmps[h]
                # -- tail chain --
                for g in range(NQB // GRP):
                    Ssb = tl.tile([128, GRP, TAIL], dt.float32, tag="Ssb")
                    for jj in range(GRP // 2):
                        stp = st_ps.tile([128, 2, TAIL], dt.float32, tag="stp")
                        for t in range(2):
                            qb = g * GRP + 2 * jj + t
                            nc.tensor.matmul(out=stp[:, t, :],
                                             lhsT=hq[:, qb * 128:(qb + 1) * 128],
                                             rhs=hk[:, KHEAD:S], start=True, stop=True)
                        nc.vector.tensor_copy(out=Ssb[:, 2 * jj:2 * jj + 2, :], in_=stp)
                    spsc = Ssb[:, :, TAIL0:TAIL]
                    gts = tl.tile([128, GRP, TAILC], dt.float32, tag="gts")
                    cfb = tl.tile([128, GRP, TAILC], dt.float16, tag="cfb")
                    mifb = tl.tile([128, GRP, TAILC], dt.float16, tag="mifb")
                    posb = tl.tile([128, GRP, TAILC], dt.float32, tag="posb")
                    wb = tl.tile([128, GRP, TAILC], dt.float32, tag="wb")
                    eqb = tl.tile([128, GRP, TAILC], dt.float16, tag="eqb")
                    mi16 = tl.tile([128, GRP, TAILC], dt.int16, tag="mi16")
                    scA = tl.tile([128, GRP, TAILC], dt.float16, tag="scA")
                    scB = tl.tile([128, GRP, TAILC], dt.float16, tag="scB")
                    c16 = tl.tile([128, GRP, TAILC], dt.int16, tag="c16")
                    gsb = tls.tile([128, GRP], dt.float32, tag="gsb")
                    # sigmoid via tanh (stays on the exp_and_others ACT table set):
                    # sigma(x) = 0.5*tanh(x/2) + 0.5
                    for j in range(GRP):
                        nc.scalar.activation(out=gts[:, j, :], in_=spsc[:, j, :], func=AF.Tanh,
                                             scale=0.5, accum_out=gsb[:, j:j + 1])
                    nc.vector.tensor_scalar(out=gts, in0=gts, scalar1=0.5, scalar2=0.5,
                                            op0=OP.mult, op1=OP.add)
                    nc.vector.tensor_scalar(out=gsb, in0=gsb, scalar1=0.5,
                                            scalar2=float(0.5 * TAILC),
                                            op0=OP.mult, op1=OP.add)
                    nc.vector.tensor_copy(out=flag_sb[:, h * NQB + g * GRP:h * NQB + (g + 1) * GRP],
                                          in_=gsb)
                    # col0 of each sub-block: g[0] - gsum (seeds the chained scan)
                    nc.vector.tensor_tensor(out=gts[:, :, 0:1], in0=gts[:, :, 0:1],
                                            in1=gsb.rearrange("p (g o) -> p g o", o=1),
                                            op=OP.subtract)
                    # chained cumsum with reset: state = rmask*state + g'
                    nc.vector.tensor_tensor_scan(
                        out=posb.rearrange("p g t -> p (g t)"),
                        data0=rmask.rearrange("p g t -> p (g t)"),
                        data1=gts.rearrange("p g t -> p (g t)"), initial=0.0,
                        op0=OP.mult, op1=OP.add)
                    # posb now holds cs - gsum (incl. own g); rev-cumsum = g - (cs - gsum)
                    # ... but col0 of gts was modified; restore contribution via using
                    # original gate value: g'[0] = g[0]-gsum, cs'[*] = cs - gsum, and
                    # pos = g - cs' works with the *original* g, so rebuild col0 first
                    nc.vector.tensor_tensor(out=gts[:, :, 0:1], in0=gts[:, :, 0:1],
                                            in1=gsb.rearrange("p (g o) -> p g o", o=1),
                                            op=OP.add)
                    nc.vector.scalar_tensor_tensor(out=posb, in0=posb, scalar=-1.0,
                                                   in1=gts, op0=OP.mult, op1=OP.add)
                    nc.vector.tensor_scalar(out=c16, in0=posb, scalar1=0.5, scalar2=62.0,
                                            op0=OP.subtract, op1=OP.min)
                    nc.vector.tensor_copy(out=cfb, in_=c16)
                    nc.vector.scalar_tensor_tensor(out=wb, in0=posb, scalar=63.0,
                                                   in1=cfb, op0=OP.min, op1=OP.subtract)
                    nc.vector.tensor_tensor(out=eqb[:, :, 1:], in0=cfb[:, :, 1:],
                                            in1=cfb[:, :, :-1], op=OP.not_equal)
                    nc.vector.memset(eqb[:, :, 0:1], 1.0)
                    nc.vector.tensor_add(out=mifb, in0=cfb, in1=offt)
                    nc.vector.scalar_tensor_tensor(out=mifb, in0=mifb, scalar=1.0,
                                                   in1=eqb, op0=OP.add, op1=OP.mult)
                    nc.vector.tensor_scalar(out=mi16, in0=mifb, scalar1=1.0, scalar2=None,
                                            op0=OP.subtract)
                    nc.vector.tensor_scalar(out=eqb, in0=eqb, scalar1=-1.0, scalar2=-1.0,
                                            op0=OP.add, op1=OP.mult)
                    ptab = tl.tile([128, GRP * NPOS], dt.int16, tag="ptab")
                    nc.gpsimd.local_scatter(out_ap=ptab[:], data_ap=iota[:], idxs_ap=mi16[:],
                                            channels=128, num_elems=GRP * NPOS,
                                            num_idxs=GRP * TAILC)
                    nc.vector.tensor_scalar(out=ptab, in0=ptab, scalar1=1, scalar2=None,
                                            op0=OP.subtract)
                    nc.gpsimd.local_scatter(out_ap=scA[:], data_ap=Tp[h][:, g * GRP:(g + 1) * GRP, :],
                                            idxs_ap=ptab[:], channels=128,
                                            num_elems=GRP * TAILC, num_idxs=GRP * NPOS)
                    nc.gpsimd.local_scatter(out_ap=scB[:], data_ap=dTp[h][:, g * GRP:(g + 1) * GRP, :],
                                            idxs_ap=ptab[:], channels=128,
                                            num_elems=GRP * TAILC, num_idxs=GRP * NPOS)
                    Ab = tl.tile([128, GRP, TAILC], dt.float32, tag="Ab")
                    Bb = tl.tile([128, GRP, TAILC], dt.float32, tag="Bb")
                    # col0 of every sub-block is a forced run-start, so one chained
                    # scan over the flattened group self-resets at block boundaries
                    nc.vector.tensor_tensor_scan(
                        out=Ab.rearrange("p g t -> p (g t)"),
                        data0=eqb.rearrange("p g t -> p (g t)"),
                        data1=scA.rearrange("p g t -> p (g t)"), initial=0.0,
                        op0=OP.mult, op1=OP.add)
                    nc.vector.tensor_tensor_scan(
                        out=Bb.rearrange("p g t -> p (g t)"),
                        data0=eqb.rearrange("p g t -> p (g t)"),
                        data1=scB.rearrange("p g t -> p (g t)"), initial=0.0,
                        op0=OP.mult, op1=OP.add)
                    nc.vector.tensor_mul(out=Bb, in0=wb, in1=Bb)
                    nc.vector.tensor_add(out=Ab, in0=Ab, in1=Bb)
                    tt_dst = tail_tmp.rearrange("p (q t) -> p q t", t=TAIL)[:, g * GRP:(g + 1) * GRP, :]
                    nc.vector.tensor_add(out=tt_dst[:, :, TAIL0:TAIL], in0=Ab, in1=spsc)
                    nc.scalar.activation(out=tt_dst[:, :, 0:TAIL0], in_=Ssb[:, :, 0:TAIL0],
                                         func=AF.Copy)
            def sct_transpose(h):
                sct = sct_pool.tile([128, 2, S], dt.bfloat16, tag="sct", name=f"sct{h}")
                scts[h] = sct
                tail_tmp = tail_tmps[h]
                # -- tail exp + transpose --
                for qb in range(NQB):
                    etile = et.tile([128, TAIL], dt.bfloat16, tag="etile")
                    nc.scalar.activation(out=etile, in_=tail_tmp[:, qb * TAIL:(qb + 1) * TAIL],
                                         func=AF.Exp)
                    pt = sct_ps.tile([128, 2, 128], dt.bfloat16, tag="sctp")
                    for t in range(2):
                        nc.tensor.transpose(pt[:, t, :], etile[:, t * 128:(t + 1) * 128], identb)
                    nc.vector.tensor_copy(out=sct.rearrange("p t (q c) -> p t q c", c=128)[:, :, qb, :],
                                          in_=pt)
            def kloop(h):
                hq = QT[h * DH:(h + 1) * DH, :]
                hk = KT[h * DH:(h + 1) * DH, :]
                sct = scts[h]
                # -- k loop: exp + PV --
                pv = pv_ps.tile([65, S], dt.float32, tag="pv")
                for kb in range(NKB):
                    if kb < NKB_HEAD:
                        sc = sc_pool.tile([128, S], dt.bfloat16, tag="sc")
                        for c in range(4):
                            sp = s_ps.tile([128, 512], dt.float32, tag="sp")
                            nc.tensor.matmul(out=sp,
                                             lhsT=hk[:, kb * 128:(kb + 1) * 128],
                                             rhs=hq[:, c * 512:(c + 1) * 512],
                                             start=True, stop=True)
                            nc.scalar.activation(out=sc[:, c * 512:(c + 1) * 512], in_=sp,
                                                 func=AF.Exp)
                    else:
                        sc = sct[:, kb - NKB_HEAD, :]
                    lhs = Vn[:, kb, 0:65] if h == 0 else Vn[:, kb, 66:131]
                    for c in range(4):
                        nc.tensor.matmul(out=pv[:, c * 512:(c + 1) * 512], lhsT=lhs,
                                         rhs=sc[:, c * 512:(c + 1) * 512],
                                         start=(kb == 0), stop=(kb == NKB - 1))
                nc.scalar.copy(out=hoT[h], in_=pv)
            def dn(h):
                # denominators for this head (shares the sct_ps slot)
                dcol = tls.tile([128, NQB], dt.float32, tag=f"dcol{h}", name=f"dcol{h}")
                for qb in range(NQB):
                    ptd = sct_ps.tile([128, 2, 128], dt.float32, tag="sctp", name=f"dnp{h}_{qb}")
                    nc.tensor.transpose(ptd[:, 0, 0:65].bitcast(dt.float32),
                                        hoT[h][:, qb * 128:(qb + 1) * 128].bitcast(dt.float32),
                                        ident32)
                    nc.scalar.copy(out=dcol[:, qb:qb + 1], in_=ptd[:, 0, 64:65].bitcast(dt.float32))
                nc.vector.reciprocal(out=rden[h], in_=dcol)
            def wo(h, evac_dve):
                for sb in range(NQB):
                    for c in range(2):
                        wop = wo_pools['ps'].tile([128, 512], dt.float32, tag="wop",
                                         name=f"wop{h}_{sb}_{c}")
                        nc.tensor.matmul(out=wop,
                                         lhsT=hoT[h][0:64, sb * 128:(sb + 1) * 128],
                                         rhs=woT[h][:, c * 512:(c + 1) * 512],
                                         start=True, stop=True)
                        ob = wo_pools['sb'].tile([128, 512], dt.float16, tag="ob",
                                        name=f"ob{h}_{sb}_{c}")
                        if evac_dve or c == 1:
                            nc.vector.tensor_scalar(out=ob, in0=wop,
                                                    scalar1=rden[h][:, sb:sb + 1],
                                                    scalar2=None, op0=OP.mult)
                        else:
                            nc.scalar.activation(out=ob, in_=wop, func=AF.Identity,
                                                 scale=rden[h][:, sb:sb + 1])
                        nc.sync.dma_start(
                            out=out_d[h, sb * 128:(sb + 1) * 128, c * 512:(c + 1) * 512],
                            in_=ob)

            with tc.tile_pool(name="st_ps", bufs=1, space="PSUM") as st_ps:
                tail_chain(0)
                sct_transpose(0)
                kloop(0)
                dn(0)
                tail_chain(1)
            with tc.tile_pool(name="wo_ps", bufs=1, space="PSUM") as _wops, \
                 tc.tile_pool(name="wo_sb", bufs=2) as _wosb:
                wo_pools['ps'], wo_pools['sb'] = _wops, _wosb
                sct_transpose(1)
                wo(0, evac_dve=True)
                kloop(1)
                dn(1)
                wo(1, evac_dve=False)

        nc.sync.dma_start(out=flag_d, in_=flag_sb)
        big.release()
        consts.release()

    nc.compile()
    return nc


def bass_ap_2range(Vn, kb):
    """out AP [128, 2, 64] covering Vn[:, kb, 0:64] and Vn[:, kb, 66:130]."""
    import concourse.bass as bass
    base = Vn[:, kb, 0:64]
    ap = [list(base.ap[0]), [66, 2], [1, 64]]
    return bass.AP(base.tensor, base.offset, [list(p) for p in ap])


def _prepare_maps(q, k, v, Wq_w, Wq_b, Wk_w, Wk_b, Wv_w, Wv_b, Wo_w, Wo_b, pos_emb):
    import ml_dtypes
    f32 = np.float32
    bf16 = ml_dtypes.bfloat16
    qT = np.ascontiguousarray(q[0].T.astype(bf16))
    kT = np.ascontiguousarray(k[0].T.astype(bf16))
    vT = np.ascontiguousarray(v[0].T.astype(bf16))
    iota = np.tile(np.arange(1, GRP * TAILC + 1, dtype=np.int16), (128, 1))
    off = np.tile(np.repeat(np.arange(GRP, dtype=np.float16) * NPOS, TAILC), (128, 1)).reshape(128, GRP, TAILC)
    identb = np.eye(128, dtype=bf16)
    ident32 = np.eye(65, dtype=f32)
    pe = np.ascontiguousarray(pos_emb.astype(bf16))
    in_maps = []
    for c in range(NCORES):
        r0 = c * DHC
        sl = slice(r0, r0 + DHC)
        in_maps.append({
            "qT": qT, "kT": kT, "vT": vT,
            "wqT": np.ascontiguousarray(Wq_w[sl, :].T.astype(bf16)),
            "wkT": np.ascontiguousarray(Wk_w[sl, :].T.astype(bf16)),
            "wvT": np.ascontiguousarray(Wv_w[sl, :].T.astype(bf16)),
            "woT": np.ascontiguousarray(Wo_w[:, sl].T.astype(f32)),
            "bq": np.ascontiguousarray(Wq_b[sl].astype(f32)[:, None]),
            "bk": np.ascontiguousarray((Wk_b[sl] * 0.125).astype(f32)[:, None]),
            "bv": np.ascontiguousarray(Wv_b[sl].astype(f32)[:, None]),
            "pe": np.concatenate([pe, pe], axis=0), "iota": iota,
            "identb": identb, "ident32": ident32, "off": np.ascontiguousarray(off),
        })
    return in_maps


def _reference_fallback(q, k, v, Wq_w, Wq_b, Wk_w, Wk_b, Wv_w, Wv_b, Wo_w, Wo_b,
                        pos_emb, nheads):
    """Exact numpy fallback (used only if the clip-safety flag fails)."""
    b, s, ndims = q.shape
    d = ndims // nheads

    def heads(x, W, bb):
        y = x.reshape(-1, ndims) @ W.T + bb
        return y.reshape(b, s, nheads, d).transpose(0, 2, 1, 3)

    Q = heads(q, Wq_w, Wq_b)
    K = heads(k, Wk_w, Wk_b)
    V = heads(v, Wv_w, Wv_b)
    logits = np.einsum("bhqd,bhkd->bhqk", Q, K) / np.sqrt(d)
    npos = pos_emb.shape[-1]
    gates = 1.0 / (1.0 + np.exp(-logits))
    pos = np.flip(np.cumsum(np.flip(gates, -1), -1), -1)
    pos = np.minimum(pos, npos - 1)
    pc = np.ceil(pos).astype(np.int64)
    pf = np.floor(pos).astype(np.int64)
    li = np.einsum("bhqd,dn->bhqn", Q, pos_emb)
    lc = np.take_along_axis(li, pc, -1)
    lf = np.take_along_axis(li, pf, -1)
    w = pos - pf
    cope = lc * w + lf * (1.0 - w)
    x = logits + cope
    x = x - x.max(-1, keepdims=True)
    e = np.exp(x)
    scores = e / e.sum(-1, keepdims=True)
    out = np.einsum("bhqk,bhkd->bhqd", scores, V)
    out = out.transpose(0, 2, 1, 3).reshape(b, s, ndims)
    return (out @ Wo_w.T + Wo_b).astype(np.float32)


def kernel(q, k, v, Wq_w, Wq_b, Wk_w, Wk_b, Wv_w, Wv_b, Wo_w, Wo_b, pos_emb,
           nheads, _want_trace=False):
    global _prog
    from concourse.bass_utils import run_bass_kernel_spmd

    q = np.asarray(q); k = np.asarray(k); v = np.asarray(v)
    Wq_w = np.asarray(Wq_w); Wq_b = np.asarray(Wq_b)
    Wk_w = np.asarray(Wk_w); Wk_b = np.asarray(Wk_b)
    Wv_w = np.asarray(Wv_w); Wv_b = np.asarray(Wv_b)
    Wo_w = np.asarray(Wo_w); Wo_b = np.asarray(Wo_b)
    pos_emb = np.asarray(pos_emb)

    if _prog is None:
        _prog = _build_program()
    in_maps = _prepare_maps(q, k, v, Wq_w, Wq_b, Wk_w, Wk_b, Wv_w, Wv_b,
                            Wo_w, Wo_b, pos_emb)
    res = run_bass_kernel_spmd(_prog, in_maps, core_ids=list(range(NCORES)),
                               trace=_want_trace)
    flag_min = min(float(r["flag"].min()) for r in res.results)
    if flag_min < float(NPOS - 1):
        out = _reference_fallback(q, k, v, Wq_w, Wq_b, Wk_w, Wk_b, Wv_w, Wv_b,
                                  Wo_w, Wo_b, pos_emb, int(nheads))
        return out if not _want_trace else (out, res)
    total = res.results[0]["out"].astype(np.float64).sum(axis=0)
    for r in res.results[1:]:
        total = total + r["out"].astype(np.float64).sum(axis=0)
    out = (total + Wo_b.astype(np.float64)).astype(np.float32)[None]
    return out if not _want_trace else (out, res)


# revision 4
# speedup vs baseline: 1.2672x; 1.0588x over previous
"""CoPE multi-head attention Trainium2 kernel.

Sharding: 16 heads / 8 cores = 2 heads per core (head/tensor parallel).
Each core gets full q,k,v (pre-transposed on host) + its head-slice of the
projection weights, computes its 2 heads' attention + its partial output
projection; host sums the 8 partials and adds the output bias.

CoPE structure exploited: pos = reverse-cumsum of sigmoid gates clips at
npos-1=63.  For keys before a 256-wide tail suffix, pos >= 63 (verified at
runtime via a flag output), so cope == T[q,63], a per-row constant that
cancels in softmax.  Only the tail needs the real interpolated gather,
done via GPSIMD local_scatter (run-start positions -> table values) + a
sample-and-hold tensor_tensor_scan.

Perf notes: all matmul operands are bf16 (FWL weight loads, half DMA);
gates use sigmoid(x) = 0.5*tanh(x/2)+0.5 so the Activation engine stays
on the exp_and_others table set; the two heads' k-loops run interleaved
(disjoint PE row-groups overlap weight loads with the other head's
matmuls) with the tail cope chains scheduled across them; the output
projection pre-normalizes PV via a transpose round-trip so both heads
share one accumulated fp16 output partial.
"""

import numpy as np

B, S, ND, NH, DH, NPOS = 1, 2048, 1024, 16, 64, 64
NCORES = 8
HPC = NH // NCORES          # heads per core = 2
DHC = HPC * DH              # head dims per core = 128
TAIL = 256                  # tail width (suffix of key axis)
KHEAD = S - TAIL            # 1792
NQB = S // 128              # 16 q blocks
NKB = S // 128              # 16 k blocks
NKB_HEAD = KHEAD // 128     # 14
GRP = 4                     # tail q-blocks per scatter group
TAILC = 192                 # columns of the tail that get the full CoPE chain
TAIL0 = TAIL - TAILC        # leading tail columns treated as clipped (delta=0)

_prog = None


def _build_program():
    import concourse.bacc as bacc
    import concourse.tile as tile
    from concourse import mybir

    dt = mybir.dt
    AF = mybir.ActivationFunctionType
    OP = mybir.AluOpType

    nc = bacc.Bacc("TRN2", target_bir_lowering=False, debug=False,
                   num_devices=NCORES)

    # ---- DRAM I/O ----
    qT_d = nc.dram_tensor("qT", [ND, S], dt.bfloat16, kind="ExternalInput").ap()
    kT_d = nc.dram_tensor("kT", [ND, S], dt.bfloat16, kind="ExternalInput").ap()
    vT_d = nc.dram_tensor("vT", [ND, S], dt.bfloat16, kind="ExternalInput").ap()
    wqT_d = nc.dram_tensor("wqT", [ND, DHC], dt.bfloat16, kind="ExternalInput").ap()
    wkT_d = nc.dram_tensor("wkT", [ND, DHC], dt.bfloat16, kind="ExternalInput").ap()
    wvT_d = nc.dram_tensor("wvT", [ND, DHC], dt.bfloat16, kind="ExternalInput").ap()
    woT_d = nc.dram_tensor("woT", [DHC, ND], dt.bfloat16, kind="ExternalInput").ap()
    bq_d = nc.dram_tensor("bq", [DHC, 1], dt.float32, kind="ExternalInput").ap()
    bk_d = nc.dram_tensor("bk", [DHC, 1], dt.float32, kind="ExternalInput").ap()  # pre-scaled by 1/8
    bv_d = nc.dram_tensor("bv", [DHC, 1], dt.float32, kind="ExternalInput").ap()
    pe_d = nc.dram_tensor("pe", [2 * DH, NPOS], dt.bfloat16, kind="ExternalInput").ap()
    iota_d = nc.dram_tensor("iota", [128, GRP * TAILC], dt.int16, kind="ExternalInput").ap()
    off_d = nc.dram_tensor("off", [128, GRP, TAILC], dt.float16, kind="ExternalInput").ap()
    identb_d = nc.dram_tensor("identb", [128, 128], dt.bfloat16, kind="ExternalInput").ap()
    ident32_d = nc.dram_tensor("ident32", [65, 65], dt.float32, kind="ExternalInput").ap()
    out_d = nc.dram_tensor("out", [S, ND], dt.float16, kind="ExternalOutput").ap()
    flag_d = nc.dram_tensor("flag", [128, HPC * NQB], dt.float32, kind="ExternalOutput").ap()

    with tile.TileContext(nc) as tc:
        consts = tc.alloc_tile_pool(name="consts", bufs=1)
        big = tc.alloc_tile_pool(name="big", bufs=1)

        # ---- constants (x-projection weights first: they gate the proj loop) ----
        wqT = consts.tile([128, 8, DHC], dt.bfloat16, tag="wq")
        wkT = consts.tile([128, 8, DHC], dt.bfloat16, tag="wk")
        wvT = consts.tile([128, 8, DHC], dt.bfloat16, tag="wv")
        nc.sync.dma_start(out=wqT, in_=wqT_d.rearrange("(b p) d -> p b d", p=128))
        nc.sync.dma_start(out=wkT, in_=wkT_d.rearrange("(b p) d -> p b d", p=128))
        nc.sync.dma_start(out=wvT, in_=wvT_d.rearrange("(b p) d -> p b d", p=128))
        bq = consts.tile([DHC, 1], dt.float32, tag="bq")
        bk = consts.tile([DHC, 1], dt.float32, tag="bk")
        bv = consts.tile([DHC, 1], dt.float32, tag="bv")
        nc.sync.dma_start(out=bq, in_=bq_d)
        nc.sync.dma_start(out=bk, in_=bk_d)
        nc.sync.dma_start(out=bv, in_=bv_d)
        identb = consts.tile([128, 128], dt.bfloat16, tag="identb")
        nc.sync.dma_start(out=identb, in_=identb_d)
        ident32 = consts.tile([65, 65], dt.float32, tag="ident32")
        nc.sync.dma_start(out=ident32, in_=ident32_d)
        pe = consts.tile([2 * DH, NPOS], dt.bfloat16, tag="pe")
        nc.sync.dma_start(out=pe, in_=pe_d)
        woT = consts.tile([DHC, ND], dt.bfloat16, tag="woT")
        iota = consts.tile([128, GRP * TAILC], dt.int16, tag="iota")
        offt = consts.tile([128, GRP, TAILC], dt.float16, tag="offt")
        rmask = consts.tile([128, GRP, TAILC], dt.float32, tag="rmask")
        nc.vector.memset(rmask, 1.0)
        nc.vector.memset(rmask[:, :, 0:1], 0.0)

        # persistent activations
        QT = big.tile([DHC, S], dt.bfloat16, tag="QT")   # [128 dh, 2048 s]
        KT = big.tile([DHC, S], dt.bfloat16, tag="KT")   # pre-scaled by 1/8
        Vn = big.tile([128, NKB, 131], dt.bfloat16, tag="Vn")  # per kblk: h0 V[0:64], ones 64, h1 V 66:130, ones 130
        flag_sb = big.tile([128, HPC * NQB], dt.float32, tag="flag")

        # ---- phase 1: projections ----
        with tc.tile_pool(name="xstage", bufs=3) as xstage, \
             tc.tile_pool(name="proj_ps", bufs=1, space="PSUM") as proj_ps, \
             tc.tile_pool(name="vt_ps", bufs=2, space="PSUM") as vt_ps, \
             tc.tile_pool(name="vstage", bufs=1) as vstage:
            VT = vstage.tile([DHC, S], dt.bfloat16, tag="VT")
            for name, x_d, wT, bias, dest, scale in (
                ("q", qT_d, wqT, bq, QT, 1.0),
                ("k", kT_d, wkT, bk, KT, 0.125),
                ("v", vT_d, wvT, bv, VT, 1.0),
            ):
                ps_chunks = [proj_ps.tile([DHC, 512], dt.float32, tag=f"pp{c}", name=f"pp_{name}_{c}") for c in range(4)]
                for db in range(8):
                    xt = xstage.tile([128, S], dt.bfloat16, tag="xt")
                    nc.sync.dma_start(out=xt, in_=x_d[db * 128:(db + 1) * 128, :])
                    for c in range(4):
                        nc.tensor.matmul(out=ps_chunks[c], lhsT=wT[:, db, :],
                                         rhs=xt[:, c * 512:(c + 1) * 512],
                                         start=(db == 0), stop=(db == 7))
                for c in range(4):
                    nc.scalar.activation(out=dest[:, c * 512:(c + 1) * 512], in_=ps_chunks[c],
                                         func=AF.Identity, bias=bias, scale=scale)
            # late consts (not needed until the tail chains / output projection)
            nc.sync.dma_start(out=woT, in_=woT_d)
            nc.sync.dma_start(out=iota, in_=iota_d)
            nc.sync.dma_start(out=offt, in_=off_d)
            # V: transpose [dh,s] -> [s,dh] per 128-block, pack into Vn with ones cols
            nc.vector.memset(Vn[:, :, 64:65], 1.0)
            nc.vector.memset(Vn[:, :, 130:131], 1.0)
            for kb in range(NKB):
                pt = vt_ps.tile([128, 128], dt.bfloat16, tag="vt")
                nc.tensor.transpose(pt, VT[:, kb * 128:(kb + 1) * 128], identb)
                dst = bass_ap_2range(Vn, kb)
                nc.vector.tensor_copy(out=dst, in_=pt.rearrange("p (b d) -> p b d", b=2))

        # ---- phase 2: cope tables T'[q,n], dT'[q,n] (fp16) per head ----
        Tp = [big.tile([128, NQB, NPOS], dt.float16, tag=f"Tp{h}", name=f"Tp{h}") for h in range(HPC)]
        dTp = [big.tile([128, NQB, NPOS], dt.float16, tag=f"dTp{h}", name=f"dTp{h}") for h in range(HPC)]
        with tc.tile_pool(name="tt_ps", bufs=2, space="PSUM") as tt_ps, \
             tc.tile_pool(name="tt_sb", bufs=2) as tt_sb:
            for h in range(HPC):
                for gg in range(2):
                    ps = tt_ps.tile([128, 8, NPOS], dt.float32, tag="ttp")
                    for qq in range(8):
                        qb = gg * 8 + qq
                        nc.tensor.matmul(out=ps[:, qq, :],
                                         lhsT=QT[h * DH:(h + 1) * DH, qb * 128:(qb + 1) * 128],
                                         rhs=pe[h * DH:(h + 1) * DH, :], start=True, stop=True)
                    tsb = tt_sb.tile([128, 8, NPOS], dt.float32, tag="tsb")
                    nc.scalar.copy(out=tsb, in_=ps)
                    for qq in range(8):
                        qb = gg * 8 + qq
                        nc.vector.tensor_scalar(out=Tp[h][:, qb, :], in0=tsb[:, qq, :],
                                                scalar1=tsb[:, qq, 63:64], scalar2=None,
                                                op0=OP.subtract)
                    nc.vector.tensor_tensor(out=dTp[h][:, gg * 8:(gg + 1) * 8, 0:63],
                                            in0=tsb[:, :, 1:64], in1=tsb[:, :, 0:63],
                                            op=OP.subtract)
                    nc.vector.memset(dTp[h][:, gg * 8:(gg + 1) * 8, 63:64], 0.0)

        # ---- phases 3-5: tail cope chains overlap a dual-head k-loop;
        # merged output projection at the end ----
        hoT = [big.tile([65, S], dt.float32r, tag=f"hoT{h}", name=f"hoT{h}") for h in range(HPC)]
        hoQ2 = big.tile([128, NQB, DHC], dt.bfloat16, tag="hoQ2")  # normalized [q, d2]
        with tc.tile_pool(name="tmp_pool", bufs=1) as tmp_pool, \
             tc.tile_pool(name="tl", bufs=2) as tl, \
             tc.tile_pool(name="tls", bufs=4) as tls, \
             tc.tile_pool(name="et", bufs=2) as et, \
             tc.tile_pool(name="sc_pool", bufs=3) as sc_pool, \
             tc.tile_pool(name="sct_pool", bufs=1) as sct_pool:
            tail_tmps = [tmp_pool.tile([128, NQB * TAIL], dt.float16, tag=f"tt{h}",
                                       name=f"tail_tmp{h}") for h in range(HPC)]
            scts = [None, None]

            def tail_group(h, g):
                hq = QT[h * DH:(h + 1) * DH, :]
                hk = KT[h * DH:(h + 1) * DH, :]
                tail_tmp = tail_tmps[h]
                Ssb = tl.tile([128, GRP, TAIL], dt.float32, tag="Ssb")
                for jj in range(GRP // 2):
                    stp = st_ps.tile([128, 2, TAIL], dt.float32, tag="stp")
                    for t in range(2):
                        qb = g * GRP + 2 * jj + t
                        nc.tensor.matmul(out=stp[:, t, :],
                                         lhsT=hq[:, qb * 128:(qb + 1) * 128],
                                         rhs=hk[:, KHEAD:S], start=True, stop=True)
                    nc.vector.tensor_copy(out=Ssb[:, 2 * jj:2 * jj + 2, :], in_=stp)
                spsc = Ssb[:, :, TAIL0:TAIL]
                gts = tl.tile([128, GRP, TAILC], dt.float32, tag="gts")
                cfb = tl.tile([128, GRP, TAILC], dt.float16, tag="cfb")
                mifb = tl.tile([128, GRP, TAILC], dt.float16, tag="mifb")
                posb = tl.tile([128, GRP, TAILC], dt.float32, tag="posb")
                wb = tl.tile([128, GRP, TAILC], dt.float16, tag="wb")
                eqb = tl.tile([128, GRP, TAILC], dt.float16, tag="eqb")
                mi16 = tl.tile([128, GRP, TAILC], dt.int16, tag="mi16")
                scA = tl.tile([128, GRP, TAILC], dt.float16, tag="scA")
                scB = tl.tile([128, GRP, TAILC], dt.float16, tag="scB")
                c16 = tl.tile([128, GRP, TAILC], dt.int16, tag="c16")
                gsb = tls.tile([128, GRP], dt.float32, tag="gsb")
                # sigmoid via tanh (stays on the exp_and_others ACT table set):
                # sigma(x) = 0.5*tanh(x/2) + 0.5
                for j in range(GRP):
                    nc.scalar.activation(out=gts[:, j, :], in_=spsc[:, j, :], func=AF.Tanh,
                                         scale=0.5, accum_out=gsb[:, j:j + 1])
                nc.vector.tensor_scalar(out=gts, in0=gts, scalar1=0.5, scalar2=0.5,
                                        op0=OP.mult, op1=OP.add)
                nc.vector.tensor_scalar(out=gsb, in0=gsb, scalar1=0.5,
                                        scalar2=float(0.5 * TAILC),
                                        op0=OP.mult, op1=OP.add)
                nc.vector.tensor_copy(out=flag_sb[:, h * NQB + g * GRP:h * NQB + (g + 1) * GRP],
                                      in_=gsb)
                # col0 of each sub-block: g[0] - gsum (seeds the chained scan)
                nc.vector.tensor_tensor(out=gts[:, :, 0:1], in0=gts[:, :, 0:1],
                                        in1=gsb.rearrange("p (g o) -> p g o", o=1),
                                        op=OP.subtract)
                # chained cumsum with reset: state = rmask*state + g'
                nc.vector.tensor_tensor_scan(
                    out=posb.rearrange("p g t -> p (g t)"),
                    data0=rmask.rearrange("p g t -> p (g t)"),
                    data1=gts.rearrange("p g t -> p (g t)"), initial=0.0,
                    op0=OP.mult, op1=OP.add)
                # posb now holds cs - gsum (incl. own g); rev-cumsum = g - (cs - gsum)
                # col0 of gts was modified for the seed; rebuild it first
                nc.vector.tensor_tensor(out=gts[:, :, 0:1], in0=gts[:, :, 0:1],
                                        in1=gsb.rearrange("p (g o) -> p g o", o=1),
                                        op=OP.add)
                nc.vector.scalar_tensor_tensor(out=posb, in0=posb, scalar=-1.0,
                                               in1=gts, op0=OP.mult, op1=OP.add)
                nc.vector.tensor_scalar(out=c16, in0=posb, scalar1=0.5, scalar2=62.0,
                                        op0=OP.subtract, op1=OP.min)
                nc.vector.tensor_copy(out=cfb, in_=c16)
                nc.vector.scalar_tensor_tensor(out=wb, in0=posb, scalar=63.0,
                                               in1=cfb, op0=OP.min, op1=OP.subtract)
                nc.vector.tensor_tensor(out=eqb[:, :, 1:], in0=cfb[:, :, 1:],
                                        in1=cfb[:, :, :-1], op=OP.not_equal)
                nc.vector.memset(eqb[:, :, 0:1], 1.0)
                nc.vector.tensor_add(out=mifb, in0=cfb, in1=offt)
                nc.vector.scalar_tensor_tensor(out=mifb, in0=mifb, scalar=1.0,
                                               in1=eqb, op0=OP.add, op1=OP.mult)
                nc.vector.tensor_scalar(out=mi16, in0=mifb, scalar1=1.0, scalar2=None,
                                        op0=OP.subtract)
                nc.vector.tensor_scalar(out=eqb, in0=eqb, scalar1=-1.0, scalar2=-1.0,
                                        op0=OP.add, op1=OP.mult)
                ptab = tl.tile([128, GRP * NPOS], dt.int16, tag="ptab")
                nc.gpsimd.local_scatter(out_ap=ptab[:], data_ap=iota[:], idxs_ap=mi16[:],
                                        channels=128, num_elems=GRP * NPOS,
                                        num_idxs=GRP * TAILC)
                nc.vector.tensor_scalar(out=ptab, in0=ptab, scalar1=1, scalar2=None,
                                        op0=OP.subtract)
                nc.gpsimd.local_scatter(out_ap=scA[:], data_ap=Tp[h][:, g * GRP:(g + 1) * GRP, :],
                                        idxs_ap=ptab[:], channels=128,
                                        num_elems=GRP * TAILC, num_idxs=GRP * NPOS)
                nc.gpsimd.local_scatter(out_ap=scB[:], data_ap=dTp[h][:, g * GRP:(g + 1) * GRP, :],
                                        idxs_ap=ptab[:], channels=128,
                                        num_elems=GRP * TAILC, num_idxs=GRP * NPOS)
                Ab = tl.tile([128, GRP, TAILC], dt.float16, tag="Ab")
                Bb = tl.tile([128, GRP, TAILC], dt.float16, tag="Bb")
                # col0 of every sub-block is a forced run-start, so one chained
                # scan over the flattened group self-resets at block boundaries
                nc.vector.tensor_tensor_scan(
                    out=Ab.rearrange("p g t -> p (g t)"),
                    data0=eqb.rearrange("p g t -> p (g t)"),
                    data1=scA.rearrange("p g t -> p (g t)"), initial=0.0,
                    op0=OP.mult, op1=OP.add)
                nc.vector.tensor_tensor_scan(
                    out=Bb.rearrange("p g t -> p (g t)"),
                    data0=eqb.rearrange("p g t -> p (g t)"),
                    data1=scB.rearrange("p g t -> p (g t)"), initial=0.0,
                    op0=OP.mult, op1=OP.add)
                nc.vector.tensor_mul(out=Bb, in0=wb, in1=Bb)
                nc.vector.tensor_add(out=Ab, in0=Ab, in1=Bb)
                tt_dst = tail_tmp.rearrange("p (q t) -> p q t", t=TAIL)[:, g * GRP:(g + 1) * GRP, :]
                nc.vector.tensor_add(out=tt_dst[:, :, TAIL0:TAIL], in0=Ab, in1=spsc)
                nc.scalar.activation(out=tt_dst[:, :, 0:TAIL0], in_=Ssb[:, :, 0:TAIL0],
                                     func=AF.Copy)

            def sct_transpose(h):
                sct = sct_pool.tile([128, 2, S], dt.bfloat16, tag=f"sct{h}", name=f"sct{h}")
                scts[h] = sct
                tail_tmp = tail_tmps[h]
                for qb in range(NQB):
                    etile = et.tile([128, TAIL], dt.bfloat16, tag="etile")
                    nc.scalar.activation(out=etile, in_=tail_tmp[:, qb * TAIL:(qb + 1) * TAIL],
                                         func=AF.Exp)
                    pt = sct_ps.tile([128, 2, 128], dt.bfloat16, tag="sctp")
                    for t in range(2):
                        nc.tensor.transpose(pt[:, t, :], etile[:, t * 128:(t + 1) * 128], identb)
                    nc.vector.tensor_copy(out=sct.rearrange("p t (q c) -> p t q c", c=128)[:, :, qb, :],
                                          in_=pt)

            def dual_kb(qh, kb, pvs):
                for h in range(HPC):
                    hq = QT[h * DH:(h + 1) * DH, :]
                    hk = KT[h * DH:(h + 1) * DH, :]
                    if kb < NKB_HEAD:
                        sc = sc_pool.tile([128, 1024], dt.bfloat16, tag="sc")
                        for cc in range(2):
                            c = 2 * qh + cc
                            sp = s_ps.tile([128, 512], dt.float32, tag="sp")
                            nc.tensor.matmul(out=sp,
                                             lhsT=hk[:, kb * 128:(kb + 1) * 128],
                                             rhs=hq[:, c * 512:(c + 1) * 512],
                                             start=True, stop=True)
                            nc.scalar.activation(out=sc[:, cc * 512:(cc + 1) * 512], in_=sp,
                                                 func=AF.Exp)
                    else:
                        sc = scts[h][:, kb - NKB_HEAD, qh * 1024:(qh + 1) * 1024]
                    lhs = Vn[:, kb, 0:65] if h == 0 else Vn[:, kb, 66:131]
                    for cc in range(2):
                        nc.tensor.matmul(out=pvs[h][:, cc * 512:(cc + 1) * 512], lhsT=lhs,
                                         rhs=sc[:, cc * 512:(cc + 1) * 512],
                                         start=(kb == 0), stop=(kb == NKB - 1))

            # schedule: tail groups interleaved with qh=0 head k-blocks
            with tc.tile_pool(name="st_ps", bufs=1, space="PSUM") as st_ps, \
                 tc.tile_pool(name="sct_ps", bufs=1, space="PSUM") as sct_ps, \
                 tc.tile_pool(name="s_ps", bufs=2, space="PSUM") as s_ps, \
                 tc.tile_pool(name="pv_ps", bufs=1, space="PSUM") as pv_ps:
                kb_sched = [[0, 1], [2, 3], [4], [5, 6], [7, 8], [9], [10, 11], [12, 13]]
                tg = [(h, g) for h in range(HPC) for g in range(4)]
                pvs = [pv_ps.tile([65, 1024], dt.float32, tag=f"pv{h}", name=f"pv_qh0_{h}")
                       for h in range(HPC)]
                for i in range(8):
                    tail_group(*tg[i])
                    for kb in kb_sched[i]:
                        dual_kb(0, kb, pvs)
                sct_transpose(0)
                sct_transpose(1)
                for kb in (NKB_HEAD, NKB_HEAD + 1):
                    dual_kb(0, kb, pvs)
                for h in range(HPC):
                    nc.scalar.copy(out=hoT[h][:, 0:1024], in_=pvs[h])
                pvs = [pv_ps.tile([65, 1024], dt.float32, tag=f"pv{h}", name=f"pv_qh1_{h}")
                       for h in range(HPC)]
                for kb in range(NKB):
                    dual_kb(1, kb, pvs)
                for h in range(HPC):
                    nc.scalar.copy(out=hoT[h][:, 1024:2048], in_=pvs[h])

            # ---- merged output projection: normalize in [q, d] space, then
            # transpose back and contract both heads at once ----
            with tc.tile_pool(name="dn_ps", bufs=2, space="PSUM") as dn_ps, \
                 tc.tile_pool(name="wo_tq", bufs=2, space="PSUM") as wo_tq, \
                 tc.tile_pool(name="wo_ps", bufs=2, space="PSUM") as wo_ps, \
                 tc.tile_pool(name="wo_sb", bufs=3) as wo_sb:
                for h in range(HPC):
                    for qb in range(NQB):
                        ptd = dn_ps.tile([128, 2, 128], dt.float32, tag="ptd",
                                         name=f"dnp{h}_{qb}")
                        nc.tensor.transpose(ptd[:, 0, 0:65].bitcast(dt.float32),
                                            hoT[h][:, qb * 128:(qb + 1) * 128].bitcast(dt.float32),
                                            ident32)
                        rcol = tls.tile([128, 1], dt.float32, tag="rcol",
                                        name=f"rcol{h}_{qb}")
                        nc.vector.reciprocal(out=rcol, in_=ptd[:, 0, 64:65])
                        nc.vector.tensor_scalar(out=hoQ2[:, qb, h * DH:(h + 1) * DH],
                                                in0=ptd[:, 0, 0:64], scalar1=rcol,
                                                scalar2=None, op0=OP.mult)
                for qb in range(NQB):
                    tq = wo_tq.tile([128, 128], dt.bfloat16, tag="tq", name=f"tq{qb}")
                    nc.tensor.transpose(tq, hoQ2[:, qb, :], identb)
                    hoTn = wo_sb.tile([128, 128], dt.bfloat16, tag="hoTn", name=f"hoTn{qb}")
                    nc.vector.tensor_copy(out=hoTn, in_=tq)
                    for c in range(2):
                        wop = wo_ps.tile([128, 512], dt.float32, tag="wop",
                                         name=f"wop{qb}_{c}")
                        nc.tensor.matmul(out=wop, lhsT=hoTn,
                                         rhs=woT[:, c * 512:(c + 1) * 512],
                                         start=True, stop=True)
                        ob = wo_sb.tile([128, 512], dt.float16, tag="ob",
                                        name=f"ob{qb}_{c}")
                        if c == 0:
                            nc.vector.tensor_copy(out=ob, in_=wop)
                        else:
                            nc.scalar.copy(out=ob, in_=wop)
                        nc.sync.dma_start(
                            out=out_d[qb * 128:(qb + 1) * 128, c * 512:(c + 1) * 512],
                            in_=ob)

        nc.sync.dma_start(out=flag_d, in_=flag_sb)
        big.release()
        consts.release()

    nc.compile()
    return nc


def bass_ap_2range(Vn, kb):
    """out AP [128, 2, 64] covering Vn[:, kb, 0:64] and Vn[:, kb, 66:130]."""
    import concourse.bass as bass
    base = Vn[:, kb, 0:64]
    ap = [list(base.ap[0]), [66, 2], [1, 64]]
    return bass.AP(base.tensor, base.offset, [list(p) for p in ap])


def _prepare_maps(q, k, v, Wq_w, Wq_b, Wk_w, Wk_b, Wv_w, Wv_b, Wo_w, Wo_b, pos_emb):
    import ml_dtypes
    f32 = np.float32
    bf16 = ml_dtypes.bfloat16
    qT = np.ascontiguousarray(q[0].T.astype(bf16))
    kT = np.ascontiguousarray(k[0].T.astype(bf16))
    vT = np.ascontiguousarray(v[0].T.astype(bf16))
    iota = np.tile(np.arange(1, GRP * TAILC + 1, dtype=np.int16), (128, 1))
    off = np.tile(np.repeat(np.arange(GRP, dtype=np.float16) * NPOS, TAILC), (128, 1)).reshape(128, GRP, TAILC)
    identb = np.eye(128, dtype=bf16)
    ident32 = np.eye(65, dtype=f32)
    pe = np.ascontiguousarray(pos_emb.astype(bf16))
    in_maps = []
    for c in range(NCORES):
        r0 = c * DHC
        sl = slice(r0, r0 + DHC)
        in_maps.append({
            "qT": qT, "kT": kT, "vT": vT,
            "wqT": np.ascontiguousarray(Wq_w[sl, :].T.astype(bf16)),
            "wkT": np.ascontiguousarray(Wk_w[sl, :].T.astype(bf16)),
            "wvT": np.ascontiguousarray(Wv_w[sl, :].T.astype(bf16)),
            "woT": np.ascontiguousarray(Wo_w[:, sl].T.astype(bf16)),
            "bq": np.ascontiguousarray(Wq_b[sl].astype(f32)[:, None]),
            "bk": np.ascontiguousarray((Wk_b[sl] * 0.125).astype(f32)[:, None]),
            "bv": np.ascontiguousarray(Wv_b[sl].astype(f32)[:, None]),
            "pe": np.concatenate([pe, pe], axis=0), "iota": iota,
            "identb": identb, "ident32": ident32, "off": np.ascontiguousarray(off),
        })
    return in_maps


def _reference_fallback(q, k, v, Wq_w, Wq_b, Wk_w, Wk_b, Wv_w, Wv_b, Wo_w, Wo_b,
                        pos_emb, nheads):
    """Exact numpy fallback (used only if the clip-safety flag fails)."""
    b, s, ndims = q.shape
    d = ndims // nheads

    def heads(x, W, bb):
        y = x.reshape(-1, ndims) @ W.T + bb
        return y.reshape(b, s, nheads, d).transpose(0, 2, 1, 3)

    Q = heads(q, Wq_w, Wq_b)
    K = heads(k, Wk_w, Wk_b)
    V = heads(v, Wv_w, Wv_b)
    logits = np.einsum("bhqd,bhkd->bhqk", Q, K) / np.sqrt(d)
    npos = pos_emb.shape[-1]
    gates = 1.0 / (1.0 + np.exp(-logits))
    pos = np.flip(np.cumsum(np.flip(gates, -1), -1), -1)
    pos = np.minimum(pos, npos - 1)
    pc = np.ceil(pos).astype(np.int64)
    pf = np.floor(pos).astype(np.int64)
    li = np.einsum("bhqd,dn->bhqn", Q, pos_emb)
    lc = np.take_along_axis(li, pc, -1)
    lf = np.take_along_axis(li, pf, -1)
    w = pos - pf
    cope = lc * w + lf * (1.0 - w)
    x = logits + cope
    x = x - x.max(-1, keepdims=True)
    e = np.exp(x)
    scores = e / e.sum(-1, keepdims=True)
    out = np.einsum("bhqk,bhkd->bhqd", scores, V)
    out = out.transpose(0, 2, 1, 3).reshape(b, s, ndims)
    return (out @ Wo_w.T + Wo_b).astype(np.float32)


def kernel(q, k, v, Wq_w, Wq_b, Wk_w, Wk_b, Wv_w, Wv_b, Wo_w, Wo_b, pos_emb,
           nheads, _want_trace=False):
    global _prog
    from concourse.bass_utils import run_bass_kernel_spmd

    q = np.asarray(q); k = np.asarray(k); v = np.asarray(v)
    Wq_w = np.asarray(Wq_w); Wq_b = np.asarray(Wq_b)
    Wk_w = np.asarray(Wk_w); Wk_b = np.asarray(Wk_b)
    Wv_w = np.asarray(Wv_w); Wv_b = np.asarray(Wv_b)
    Wo_w = np.asarray(Wo_w); Wo_b = np.asarray(Wo_b)
    pos_emb = np.asarray(pos_emb)

    if _prog is None:
        _prog = _build_program()
    in_maps = _prepare_maps(q, k, v, Wq_w, Wq_b, Wk_w, Wk_b, Wv_w, Wv_b,
                            Wo_w, Wo_b, pos_emb)
    res = run_bass_kernel_spmd(_prog, in_maps, core_ids=list(range(NCORES)),
                               trace=_want_trace)
    flag_min = min(float(r["flag"].min()) for r in res.results)
    if flag_min < float(NPOS - 1):
        out = _reference_fallback(q, k, v, Wq_w, Wq_b, Wk_w, Wk_b, Wv_w, Wv_b,
                                  Wo_w, Wo_b, pos_emb, int(nheads))
        return out if not _want_trace else (out, res)
    total = res.results[0]["out"].astype(np.float64)
    for r in res.results[1:]:
        total = total + r["out"].astype(np.float64)
    out = (total + Wo_b.astype(np.float64)).astype(np.float32)[None]
    return out if not _want_trace else (out, res)


# revision 9
# speedup vs baseline: 1.4111x; 1.1135x over previous
"""CoPE multi-head attention Trainium2 kernel.

Sharding: 16 heads / 8 cores = 2 heads per core (head/tensor parallel).
Each core gets full q,k,v (pre-transposed on host) + its head-slice of the
projection weights, computes its 2 heads' attention + its partial output
projection; host sums the 8 partials and adds the output bias.

CoPE structure exploited: pos = reverse-cumsum of sigmoid gates clips at
npos-1=63.  For keys before a 256-wide tail suffix, pos >= 63 (verified at
runtime via a flag output), so cope == T[q,63], a per-row constant that
cancels in softmax.  Only the tail needs the real interpolated gather,
done via GPSIMD local_scatter (run-start positions -> table values) + a
sample-and-hold tensor_tensor_scan.

Perf notes: all matmul operands are bf16 (FWL weight loads, half DMA);
gates use sigmoid(x) = 0.5*tanh(x/2)+0.5 so the Activation engine stays
on the exp_and_others table set; the two heads' k-loops run interleaved
(disjoint PE row-groups overlap weight loads with the other head's
matmuls) with the tail cope chains scheduled across them; the output
projection pre-normalizes PV via a transpose round-trip so both heads
share one accumulated fp16 output partial.
"""

import numpy as np

B, S, ND, NH, DH, NPOS = 1, 2048, 1024, 16, 64, 64
NCORES = 8
HPC = NH // NCORES          # heads per core = 2
DHC = HPC * DH              # head dims per core = 128
TAIL = 256                  # tail width (suffix of key axis)
KHEAD = S - TAIL            # 1792
NQB = S // 128              # 16 q blocks
NKB = S // 128              # 16 k blocks
NKB_HEAD = KHEAD // 128     # 14
GRP = 4                     # tail q-blocks per scatter group
TAILC = 192                 # columns of the tail that get the full CoPE chain
TAIL0 = TAIL - TAILC        # leading tail columns treated as clipped (delta=0)

_prog = None


def _build_program():
    import concourse.bacc as bacc
    import concourse.tile as tile
    from concourse import mybir

    dt = mybir.dt
    AF = mybir.ActivationFunctionType
    OP = mybir.AluOpType

    nc = bacc.Bacc("TRN2", target_bir_lowering=False, debug=False,
                   num_devices=NCORES)

    # ---- DRAM I/O ----
    qT_d = nc.dram_tensor("qT", [ND, S], dt.bfloat16, kind="ExternalInput").ap()
    kT_d = nc.dram_tensor("kT", [ND, S], dt.bfloat16, kind="ExternalInput").ap()
    vT_d = nc.dram_tensor("vT", [ND, S], dt.bfloat16, kind="ExternalInput").ap()
    wqT_d = nc.dram_tensor("wqT", [ND, DHC], dt.bfloat16, kind="ExternalInput").ap()
    wkT_d = nc.dram_tensor("wkT", [ND, DHC], dt.bfloat16, kind="ExternalInput").ap()
    wvT_d = nc.dram_tensor("wvT", [ND, DHC], dt.bfloat16, kind="ExternalInput").ap()
    woT_d = nc.dram_tensor("woT", [DHC, ND], dt.bfloat16, kind="ExternalInput").ap()
    bq_d = nc.dram_tensor("bq", [DHC, 1], dt.float32, kind="ExternalInput").ap()
    bk_d = nc.dram_tensor("bk", [DHC, 1], dt.float32, kind="ExternalInput").ap()  # pre-scaled by 1/8
    bv_d = nc.dram_tensor("bv", [DHC, 1], dt.float32, kind="ExternalInput").ap()
    pe_d = nc.dram_tensor("pe", [2 * DH, NPOS], dt.bfloat16, kind="ExternalInput").ap()
    iota_d = nc.dram_tensor("iota", [128, GRP * TAILC], dt.int16, kind="ExternalInput").ap()
    off_d = nc.dram_tensor("off", [128, GRP, TAILC], dt.float16, kind="ExternalInput").ap()
    identb_d = nc.dram_tensor("identb", [128, 128], dt.bfloat16, kind="ExternalInput").ap()
    ident32_d = nc.dram_tensor("ident32", [65, 65], dt.float32, kind="ExternalInput").ap()
    out_d = nc.dram_tensor("out", [S, ND], dt.float16, kind="ExternalOutput").ap()
    flag_d = nc.dram_tensor("flag", [128, HPC * NQB], dt.float32, kind="ExternalOutput").ap()

    with tile.TileContext(nc) as tc:
        consts = tc.alloc_tile_pool(name="consts", bufs=1)
        big = tc.alloc_tile_pool(name="big", bufs=1)

        # ---- constants (DMAs staged through the flow; tiles declared here) ----
        wqT = consts.tile([128, 8, DHC], dt.bfloat16, tag="wq")
        wkT = consts.tile([128, 8, DHC], dt.bfloat16, tag="wk")
        wvT = consts.tile([128, 8, DHC], dt.bfloat16, tag="wv")
        bq = consts.tile([DHC, 1], dt.float32, tag="bq")
        bk = consts.tile([DHC, 1], dt.float32, tag="bk")
        bv = consts.tile([DHC, 1], dt.float32, tag="bv")
        identb = consts.tile([128, 128], dt.bfloat16, tag="identb")
        ident32 = consts.tile([65, 65], dt.float32, tag="ident32")
        pe = consts.tile([2 * DH, NPOS], dt.bfloat16, tag="pe")
        woT = consts.tile([DHC, ND], dt.bfloat16, tag="woT")
        iota = consts.tile([128, GRP * TAILC], dt.int16, tag="iota")
        offt = consts.tile([128, GRP, TAILC], dt.float16, tag="offt")
        rmask = consts.tile([128, GRP, TAILC], dt.float32, tag="rmask")
        nc.vector.memset(rmask, 1.0)
        nc.vector.memset(rmask[:, :, 0:1], 0.0)

        # persistent activations
        QT = big.tile([DHC, S], dt.bfloat16, tag="QT")   # [128 dh, 2048 s]
        KT = big.tile([DHC, S], dt.bfloat16, tag="KT")   # pre-scaled by 1/8
        Vn = big.tile([128, NKB, 131], dt.bfloat16, tag="Vn")  # per kblk: h0 V[0:64], ones 64, h1 V 66:130, ones 130
        flag_sb = big.tile([128, HPC * NQB], dt.float32, tag="flag")

        # ---- phase 1: projections ----
        with tc.tile_pool(name="xstage", bufs=3) as xstage, \
             tc.tile_pool(name="proj_ps", bufs=1, space="PSUM") as proj_ps, \
             tc.tile_pool(name="vt_ps", bufs=2, space="PSUM") as vt_ps, \
             tc.tile_pool(name="vstage", bufs=1) as vstage:
            VT = vstage.tile([DHC, S], dt.bfloat16, tag="VT")
            nc.sync.dma_start(out=wqT, in_=wqT_d.rearrange("(b p) d -> p b d", p=128))
            nc.sync.dma_start(out=bq, in_=bq_d)
            nc.sync.dma_start(out=wkT, in_=wkT_d.rearrange("(b p) d -> p b d", p=128))
            nc.sync.dma_start(out=bk, in_=bk_d)
            nc.sync.dma_start(out=wvT, in_=wvT_d.rearrange("(b p) d -> p b d", p=128))
            nc.sync.dma_start(out=bv, in_=bv_d)
            nc.sync.dma_start(out=identb, in_=identb_d)
            for name, x_d, wT, bias, dest, scale in (
                ("q", qT_d, wqT, bq, QT, 1.0),
                ("k", kT_d, wkT, bk, KT, 0.125),
                ("v", vT_d, wvT, bv, VT, 1.0),
            ):
                ps_chunks = [proj_ps.tile([DHC, 512], dt.float32, tag=f"pp{c}", name=f"pp_{name}_{c}") for c in range(4)]
                for db in range(8):
                    xt = xstage.tile([128, S], dt.bfloat16, tag="xt")
                    nc.sync.dma_start(out=xt, in_=x_d[db * 128:(db + 1) * 128, :])
                    for c in range(4):
                        nc.tensor.matmul(out=ps_chunks[c], lhsT=wT[:, db, :],
                                         rhs=xt[:, c * 512:(c + 1) * 512],
                                         start=(db == 0), stop=(db == 7))
                for c in range(4):
                    nc.vector.tensor_scalar(out=dest[:, c * 512:(c + 1) * 512],
                                            in0=ps_chunks[c], scalar1=scale,
                                            scalar2=bias, op0=OP.mult, op1=OP.add)
                if name == "q":
                    nc.sync.dma_start(out=pe, in_=pe_d)
            # late consts (not needed until the tail chains / output projection)
            nc.sync.dma_start(out=woT, in_=woT_d)
            nc.sync.dma_start(out=iota, in_=iota_d)
            nc.sync.dma_start(out=offt, in_=off_d)
            nc.sync.dma_start(out=ident32, in_=ident32_d)
            # V: transpose [dh,s] -> [s,dh] per 128-block, pack into Vn with ones cols
            nc.vector.memset(Vn[:, :, 64:65], 1.0)
            nc.vector.memset(Vn[:, :, 130:131], 1.0)
            for kb in range(NKB):
                pt = vt_ps.tile([128, 128], dt.bfloat16, tag="vt")
                nc.tensor.transpose(pt, VT[:, kb * 128:(kb + 1) * 128], identb)
                dst = bass_ap_2range(Vn, kb)
                nc.vector.tensor_copy(out=dst, in_=pt.rearrange("p (b d) -> p b d", b=2))

        # ---- phase 2: cope tables T'[q,n], dT'[q,n] (fp16) per head ----
        Tp = [big.tile([128, NQB, NPOS], dt.float16, tag=f"Tp{h}", name=f"Tp{h}") for h in range(HPC)]
        dTp = [big.tile([128, NQB, NPOS], dt.float16, tag=f"dTp{h}", name=f"dTp{h}") for h in range(HPC)]
        with tc.tile_pool(name="tt_ps", bufs=2, space="PSUM") as tt_ps, \
             tc.tile_pool(name="tt_sb", bufs=2) as tt_sb:
            for h in range(HPC):
                for gg in range(2):
                    ps = tt_ps.tile([128, 8, NPOS], dt.float32, tag="ttp")
                    for qq in range(8):
                        qb = gg * 8 + qq
                        nc.tensor.matmul(out=ps[:, qq, :],
                                         lhsT=QT[h * DH:(h + 1) * DH, qb * 128:(qb + 1) * 128],
                                         rhs=pe[h * DH:(h + 1) * DH, :], start=True, stop=True)
                    tsb = tt_sb.tile([128, 8, NPOS], dt.float32, tag="tsb")
                    nc.scalar.copy(out=tsb, in_=ps)
                    for qq in range(8):
                        qb = gg * 8 + qq
                        nc.vector.tensor_scalar(out=Tp[h][:, qb, :], in0=tsb[:, qq, :],
                                                scalar1=tsb[:, qq, 63:64], scalar2=None,
                                                op0=OP.subtract)
                    nc.vector.tensor_tensor(out=dTp[h][:, gg * 8:(gg + 1) * 8, 0:63],
                                            in0=tsb[:, :, 1:64], in1=tsb[:, :, 0:63],
                                            op=OP.subtract)
                    nc.vector.memset(dTp[h][:, gg * 8:(gg + 1) * 8, 63:64], 0.0)

        # ---- phases 3-5: tail cope chains overlap a dual-head k-loop;
        # merged output projection at the end ----
        hoT = [big.tile([65, S], dt.float32r, tag=f"hoT{h}", name=f"hoT{h}") for h in range(HPC)]
        hoQ2 = big.tile([128, NQB, DHC], dt.bfloat16, tag="hoQ2")  # normalized [q, d2]
        with tc.tile_pool(name="tmp_pool", bufs=1) as tmp_pool, \
             tc.tile_pool(name="tl", bufs=2) as tl, \
             tc.tile_pool(name="tls", bufs=4) as tls, \
             tc.tile_pool(name="et", bufs=2) as et, \
             tc.tile_pool(name="sc_pool", bufs=4) as sc_pool, \
             tc.tile_pool(name="sct_pool", bufs=1) as sct_pool:
            tail_tmps = [tmp_pool.tile([128, NQB * TAIL], dt.float16, tag=f"tt{h}",
                                       name=f"tail_tmp{h}") for h in range(HPC)]
            scts = [None, None]

            def tail_group(h, g):
                hq = QT[h * DH:(h + 1) * DH, :]
                hk = KT[h * DH:(h + 1) * DH, :]
                tail_tmp = tail_tmps[h]
                Ssb = tl.tile([128, GRP, TAIL], dt.float16, tag="Ssb")
                for jj in range(GRP // 2):
                    stp = st_ps.tile([128, 2, TAIL], dt.float32, tag="stp")
                    for t in range(2):
                        qb = g * GRP + 2 * jj + t
                        nc.tensor.matmul(out=stp[:, t, :],
                                         lhsT=hq[:, qb * 128:(qb + 1) * 128],
                                         rhs=hk[:, KHEAD:S], start=True, stop=True)
                    nc.vector.tensor_copy(out=Ssb[:, 2 * jj:2 * jj + 2, :], in_=stp)
                spsc = Ssb[:, :, TAIL0:TAIL]
                gts = tl.tile([128, GRP, TAILC], dt.float32, tag="gts")
                cfb = tl.tile([128, GRP, TAILC], dt.float16, tag="cfb")
                mifb = tl.tile([128, GRP, TAILC], dt.float16, tag="mifb")
                posb = tl.tile([128, GRP, TAILC], dt.float32, tag="posb")
                wb = tl.tile([128, GRP, TAILC], dt.float16, tag="wb")
                eqb = tl.tile([128, GRP, TAILC], dt.float16, tag="eqb")
                mi16 = tl.tile([128, GRP, TAILC], dt.int16, tag="mi16")
                scA = tl.tile([128, GRP, TAILC], dt.float16, tag="scA")
                scB = tl.tile([128, GRP, TAILC], dt.float16, tag="scB")
                c16 = tl.tile([128, GRP, TAILC], dt.int16, tag="c16")
                gsb = tls.tile([128, GRP], dt.float32, tag="gsb")
                # sigmoid via tanh (stays on the exp_and_others ACT table set):
                # sigma(x) = 0.5*tanh(x/2) + 0.5
                for j in range(GRP):
                    nc.scalar.activation(out=gts[:, j, :], in_=spsc[:, j, :], func=AF.Tanh,
                                         scale=0.5, accum_out=gsb[:, j:j + 1])
                nc.vector.tensor_scalar(out=gts, in0=gts, scalar1=0.5, scalar2=0.5,
                                        op0=OP.mult, op1=OP.add)
                nc.vector.tensor_scalar(out=gsb, in0=gsb, scalar1=0.5,
                                        scalar2=float(0.5 * TAILC),
                                        op0=OP.mult, op1=OP.add)
                nc.vector.tensor_copy(out=flag_sb[:, h * NQB + g * GRP:h * NQB + (g + 1) * GRP],
                                      in_=gsb)
                # col0 of each sub-block: g[0] - gsum (seeds the chained scan)
                nc.vector.tensor_tensor(out=gts[:, :, 0:1], in0=gts[:, :, 0:1],
                                        in1=gsb.rearrange("p (g o) -> p g o", o=1),
                                        op=OP.subtract)
                # chained cumsum with reset: state = rmask*state + g'
                nc.vector.tensor_tensor_scan(
                    out=posb.rearrange("p g t -> p (g t)"),
                    data0=rmask.rearrange("p g t -> p (g t)"),
                    data1=gts.rearrange("p g t -> p (g t)"), initial=0.0,
                    op0=OP.mult, op1=OP.add)
                # posb now holds cs - gsum (incl. own g); rev-cumsum = g - (cs - gsum)
                # col0 of gts was modified for the seed; rebuild it first
                nc.vector.tensor_tensor(out=gts[:, :, 0:1], in0=gts[:, :, 0:1],
                                        in1=gsb.rearrange("p (g o) -> p g o", o=1),
                                        op=OP.add)
                nc.vector.scalar_tensor_tensor(out=posb, in0=posb, scalar=-1.0,
                                               in1=gts, op0=OP.mult, op1=OP.add)
                nc.vector.tensor_scalar(out=c16, in0=posb, scalar1=0.5, scalar2=62.0,
                                        op0=OP.subtract, op1=OP.min)
                nc.vector.tensor_copy(out=cfb, in_=c16)
                nc.vector.scalar_tensor_tensor(out=wb, in0=posb, scalar=63.0,
                                               in1=cfb, op0=OP.min, op1=OP.subtract)
                nc.vector.tensor_tensor(out=eqb[:, :, 1:], in0=cfb[:, :, 1:],
                                        in1=cfb[:, :, :-1], op=OP.not_equal)
                nc.vector.memset(eqb[:, :, 0:1], 1.0)
                nc.vector.tensor_add(out=mifb, in0=cfb, in1=offt)
                nc.vector.scalar_tensor_tensor(out=mifb, in0=mifb, scalar=1.0,
                                               in1=eqb, op0=OP.add, op1=OP.mult)
                nc.vector.tensor_scalar(out=mi16, in0=mifb, scalar1=1.0, scalar2=None,
                                        op0=OP.subtract)
                nc.vector.tensor_scalar(out=eqb, in0=eqb, scalar1=-1.0, scalar2=-1.0,
                                        op0=OP.add, op1=OP.mult)
                ptab = tl.tile([128, GRP * NPOS], dt.int16, tag="ptab")
                nc.gpsimd.local_scatter(out_ap=ptab[:], data_ap=iota[:], idxs_ap=mi16[:],
                                        channels=128, num_elems=GRP * NPOS,
                                        num_idxs=GRP * TAILC)
                nc.vector.tensor_scalar(out=ptab, in0=ptab, scalar1=1, scalar2=None,
                                        op0=OP.subtract)
                nc.gpsimd.local_scatter(out_ap=scA[:], data_ap=Tp[h][:, g * GRP:(g + 1) * GRP, :],
                                        idxs_ap=ptab[:], channels=128,
                                        num_elems=GRP * TAILC, num_idxs=GRP * NPOS)
                nc.gpsimd.local_scatter(out_ap=scB[:], data_ap=dTp[h][:, g * GRP:(g + 1) * GRP, :],
                                        idxs_ap=ptab[:], channels=128,
                                        num_elems=GRP * TAILC, num_idxs=GRP * NPOS)
                Ab = tl.tile([128, GRP, TAILC], dt.float16, tag="Ab")
                Bb = tl.tile([128, GRP, TAILC], dt.float16, tag="Bb")
                # col0 of every sub-block is a forced run-start, so one chained
                # scan over the flattened group self-resets at block boundaries
                nc.vector.tensor_tensor_scan(
                    out=Ab.rearrange("p g t -> p (g t)"),
                    data0=eqb.rearrange("p g t -> p (g t)"),
                    data1=scA.rearrange("p g t -> p (g t)"), initial=0.0,
                    op0=OP.mult, op1=OP.add)
                nc.vector.tensor_tensor_scan(
                    out=Bb.rearrange("p g t -> p (g t)"),
                    data0=eqb.rearrange("p g t -> p (g t)"),
                    data1=scB.rearrange("p g t -> p (g t)"), initial=0.0,
                    op0=OP.mult, op1=OP.add)
                nc.vector.tensor_mul(out=Bb, in0=wb, in1=Bb)
                nc.vector.tensor_add(out=Ab, in0=Ab, in1=Bb)
                tt_dst = tail_tmp.rearrange("p (q t) -> p q t", t=TAIL)[:, g * GRP:(g + 1) * GRP, :]
                nc.vector.tensor_add(out=tt_dst[:, :, TAIL0:TAIL], in0=Ab, in1=spsc)
                nc.scalar.activation(out=tt_dst[:, :, 0:TAIL0], in_=Ssb[:, :, 0:TAIL0],
                                     func=AF.Copy)

            def sct_piece(h, g):
                # exp + transpose the tail logits for this group's 4 q-blocks;
                # one batched PSUM tile + one copy keeps the PE stream dense
                if scts[h] is None:
                    scts[h] = sct_pool.tile([128, 2, S], dt.bfloat16, tag=f"sct{h}",
                                            name=f"sct{h}")
                sct = scts[h]
                tail_tmp = tail_tmps[h]
                pt = sct_ps.tile([128, 2, GRP * 128], dt.bfloat16, tag="sctp",
                                 name=f"sctp{h}_{g}")
                for j in range(GRP):
                    qb = g * GRP + j
                    etile = et.tile([128, TAIL], dt.bfloat16, tag="etile")
                    nc.scalar.activation(out=etile, in_=tail_tmp[:, qb * TAIL:(qb + 1) * TAIL],
                                         func=AF.Exp)
                    for t in range(2):
                        nc.tensor.transpose(pt[:, t, j * 128:(j + 1) * 128],
                                            etile[:, t * 128:(t + 1) * 128], identb)
                nc.vector.tensor_copy(
                    out=sct.rearrange("p t (g c) -> p t g c", c=GRP * 128)[:, :, g, :],
                    in_=pt)

            def emit_qk(qh, kb):
                out = []
                for h in range(HPC):
                    hq = QT[h * DH:(h + 1) * DH, :]
                    hk = KT[h * DH:(h + 1) * DH, :]
                    if kb < NKB_HEAD:
                        sc = sc_pool.tile([128, 1024], dt.bfloat16, tag="sc")
                        for cc in range(2):
                            c = 2 * qh + cc
                            sp = s_ps.tile([128, 512], dt.float32, tag="sp")
                            nc.tensor.matmul(out=sp,
                                             lhsT=hk[:, kb * 128:(kb + 1) * 128],
                                             rhs=hq[:, c * 512:(c + 1) * 512],
                                             start=True, stop=True)
                            nc.scalar.activation(out=sc[:, cc * 512:(cc + 1) * 512], in_=sp,
                                                 func=AF.Exp)
                    else:
                        sc = scts[h][:, kb - NKB_HEAD, qh * 1024:(qh + 1) * 1024]
                    out.append((h, kb, sc))
                return out

            def emit_pv(entries, pvs):
                for h, kb, sc in entries:
                    lhs = Vn[:, kb, 0:65] if h == 0 else Vn[:, kb, 66:131]
                    for cc in range(2):
                        nc.tensor.matmul(out=pvs[h][:, cc * 512:(cc + 1) * 512], lhsT=lhs,
                                         rhs=sc[:, cc * 512:(cc + 1) * 512],
                                         start=(kb == 0), stop=(kb == NKB - 1))

            # schedule: tail groups + their sct pieces interleaved with qh=0
            # k-blocks; PV runs one kb behind QK so the exp latency is hidden
            with tc.tile_pool(name="st_ps", bufs=1, space="PSUM") as st_ps, \
                 tc.tile_pool(name="sct_ps", bufs=1, space="PSUM") as sct_ps, \
                 tc.tile_pool(name="s_ps", bufs=2, space="PSUM") as s_ps, \
                 tc.tile_pool(name="pv_ps", bufs=1, space="PSUM") as pv_ps:
                kb_sched = [[0, 1], [2, 3], [4], [5, 6], [7, 8], [9], [10, 11], [12, 13]]
                tg = [(h, g) for h in range(HPC) for g in range(4)]
                pvs = [pv_ps.tile([65, 1024], dt.float32, tag=f"pv{h}", name=f"pv_qh0_{h}")
                       for h in range(HPC)]
                pend = []
                for i in range(8):
                    tail_group(*tg[i])
                    sct_piece(*tg[i])
                    for kb in kb_sched[i]:
                        cur = emit_qk(0, kb)
                        emit_pv(pend, pvs)
                        pend = cur
                for kb in (NKB_HEAD, NKB_HEAD + 1):
                    cur = emit_qk(0, kb)
                    emit_pv(pend, pvs)
                    pend = cur
                emit_pv(pend, pvs)
                pend = []
                for h in range(HPC):
                    nc.scalar.copy(out=hoT[h][:, 0:1024], in_=pvs[h])
                pvs = [pv_ps.tile([65, 1024], dt.float32, tag=f"pv{h}", name=f"pv_qh1_{h}")
                       for h in range(HPC)]
                for kb in range(NKB):
                    cur = emit_qk(1, kb)
                    emit_pv(pend, pvs)
                    pend = cur
                emit_pv(pend, pvs)
                for h in range(HPC):
                    nc.scalar.copy(out=hoT[h][:, 1024:2048], in_=pvs[h])

            # ---- merged output projection: normalize in [q, d] space, then
            # transpose back and contract both heads at once ----
            with tc.tile_pool(name="dn_ps", bufs=2, space="PSUM") as dn_ps, \
                 tc.tile_pool(name="wo_tq", bufs=2, space="PSUM") as wo_tq, \
                 tc.tile_pool(name="wo_ps", bufs=2, space="PSUM") as wo_ps, \
                 tc.tile_pool(name="wo_sb", bufs=3) as wo_sb:
                for h in range(HPC):
                    for qb in range(NQB):
                        ptd = dn_ps.tile([128, 2, 128], dt.float32, tag="ptd",
                                         name=f"dnp{h}_{qb}")
                        nc.tensor.transpose(ptd[:, 0, 0:65].bitcast(dt.float32),
                                            hoT[h][:, qb * 128:(qb + 1) * 128].bitcast(dt.float32),
                                            ident32)
                        rcol = tls.tile([128, 1], dt.float32, tag="rcol",
                                        name=f"rcol{h}_{qb}")
                        nc.vector.reciprocal(out=rcol, in_=ptd[:, 0, 64:65])
                        nc.vector.tensor_scalar(out=hoQ2[:, qb, h * DH:(h + 1) * DH],
                                                in0=ptd[:, 0, 0:64], scalar1=rcol,
                                                scalar2=None, op0=OP.mult)
                for qb in range(NQB):
                    tq = wo_tq.tile([128, 128], dt.bfloat16, tag="tq", name=f"tq{qb}")
                    nc.tensor.transpose(tq, hoQ2[:, qb, :], identb)
                    hoTn = wo_sb.tile([128, 128], dt.bfloat16, tag="hoTn", name=f"hoTn{qb}")
                    nc.vector.tensor_copy(out=hoTn, in_=tq)
                    for c in range(2):
                        wop = wo_ps.tile([128, 512], dt.float32, tag="wop",
                                         name=f"wop{qb}_{c}")
                        nc.tensor.matmul(out=wop, lhsT=hoTn,
                                         rhs=woT[:, c * 512:(c + 1) * 512],
                                         start=True, stop=True)
                        ob = wo_sb.tile([128, 512], dt.float16, tag="ob",
                                        name=f"ob{qb}_{c}")
                        if c == 0:
                            nc.vector.tensor_copy(out=ob, in_=wop)
                        else:
                            nc.scalar.copy(out=ob, in_=wop)
                        nc.sync.dma_start(
                            out=out_d[qb * 128:(qb + 1) * 128, c * 512:(c + 1) * 512],
                            in_=ob)

        nc.sync.dma_start(out=flag_d, in_=flag_sb)
        big.release()
        consts.release()

    nc.compile()
    return nc


def bass_ap_2range(Vn, kb):
    """out AP [128, 2, 64] covering Vn[:, kb, 0:64] and Vn[:, kb, 66:130]."""
    import concourse.bass as bass
    base = Vn[:, kb, 0:64]
    ap = [list(base.ap[0]), [66, 2], [1, 64]]
    return bass.AP(base.tensor, base.offset, [list(p) for p in ap])


def _prepare_maps(q, k, v, Wq_w, Wq_b, Wk_w, Wk_b, Wv_w, Wv_b, Wo_w, Wo_b, pos_emb):
    import ml_dtypes
    f32 = np.float32
    bf16 = ml_dtypes.bfloat16
    qT = np.ascontiguousarray(q[0].T.astype(bf16))
    kT = np.ascontiguousarray(k[0].T.astype(bf16))
    vT = np.ascontiguousarray(v[0].T.astype(bf16))
    iota = np.tile(np.arange(1, GRP * TAILC + 1, dtype=np.int16), (128, 1))
    off = np.tile(np.repeat(np.arange(GRP, dtype=np.float16) * NPOS, TAILC), (128, 1)).reshape(128, GRP, TAILC)
    identb = np.eye(128, dtype=bf16)
    ident32 = np.eye(65, dtype=f32)
    pe = np.ascontiguousarray(pos_emb.astype(bf16))
    in_maps = []
    for c in range(NCORES):
        r0 = c * DHC
        sl = slice(r0, r0 + DHC)
        in_maps.append({
            "qT": qT, "kT": kT, "vT": vT,
            "wqT": np.ascontiguousarray(Wq_w[sl, :].T.astype(bf16)),
            "wkT": np.ascontiguousarray(Wk_w[sl, :].T.astype(bf16)),
            "wvT": np.ascontiguousarray(Wv_w[sl, :].T.astype(bf16)),
            "woT": np.ascontiguousarray(Wo_w[:, sl].T.astype(bf16)),
            "bq": np.ascontiguousarray(Wq_b[sl].astype(f32)[:, None]),
            "bk": np.ascontiguousarray((Wk_b[sl] * 0.125).astype(f32)[:, None]),
            "bv": np.ascontiguousarray(Wv_b[sl].astype(f32)[:, None]),
            "pe": np.concatenate([pe, pe], axis=0), "iota": iota,
            "identb": identb, "ident32": ident32, "off": np.ascontiguousarray(off),
        })
    return in_maps


def _reference_fallback(q, k, v, Wq_w, Wq_b, Wk_w, Wk_b, Wv_w, Wv_b, Wo_w, Wo_b,
                        pos_emb, nheads):
    """Exact numpy fallback (used only if the clip-safety flag fails)."""
    b, s, ndims = q.shape
    d = ndims // nheads

    def heads(x, W, bb):
        y = x.reshape(-1, ndims) @ W.T + bb
        return y.reshape(b, s, nheads, d).transpose(0, 2, 1, 3)

    Q = heads(q, Wq_w, Wq_b)
    K = heads(k, Wk_w, Wk_b)
    V = heads(v, Wv_w, Wv_b)
    logits = np.einsum("bhqd,bhkd->bhqk", Q, K) / np.sqrt(d)
    npos = pos_emb.shape[-1]
    gates = 1.0 / (1.0 + np.exp(-logits))
    pos = np.flip(np.cumsum(np.flip(gates, -1), -1), -1)
    pos = np.minimum(pos, npos - 1)
    pc = np.ceil(pos).astype(np.int64)
    pf = np.floor(pos).astype(np.int64)
    li = np.einsum("bhqd,dn->bhqn", Q, pos_emb)
    lc = np.take_along_axis(li, pc, -1)
    lf = np.take_along_axis(li, pf, -1)
    w = pos - pf
    cope = lc * w + lf * (1.0 - w)
    x = logits + cope
    x = x - x.max(-1, keepdims=True)
    e = np.exp(x)
    scores = e / e.sum(-1, keepdims=True)
    out = np.einsum("bhqk,bhkd->bhqd", scores, V)
    out = out.transpose(0, 2, 1, 3).reshape(b, s, ndims)
    return (out @ Wo_w.T + Wo_b).astype(np.float32)


def kernel(q, k, v, Wq_w, Wq_b, Wk_w, Wk_b, Wv_w, Wv_b, Wo_w, Wo_b, pos_emb,
           nheads, _want_trace=False):
    global _prog
    from concourse.bass_utils import run_bass_kernel_spmd

    q = np.asarray(q); k = np.asarray(k); v = np.asarray(v)
    Wq_w = np.asarray(Wq_w); Wq_b = np.asarray(Wq_b)
    Wk_w = np.asarray(Wk_w); Wk_b = np.asarray(Wk_b)
    Wv_w = np.asarray(Wv_w); Wv_b = np.asarray(Wv_b)
    Wo_w = np.asarray(Wo_w); Wo_b = np.asarray(Wo_b)
    pos_emb = np.asarray(pos_emb)

    if _prog is None:
        _prog = _build_program()
    in_maps = _prepare_maps(q, k, v, Wq_w, Wq_b, Wk_w, Wk_b, Wv_w, Wv_b,
                            Wo_w, Wo_b, pos_emb)
    res = run_bass_kernel_spmd(_prog, in_maps, core_ids=list(range(NCORES)),
                               trace=_want_trace)
    flag_min = min(float(r["flag"].min()) for r in res.results)
    if flag_min < float(NPOS - 1):
        out = _reference_fallback(q, k, v, Wq_w, Wq_b, Wk_w, Wk_b, Wv_w, Wv_b,
                                  Wo_w, Wo_b, pos_emb, int(nheads))
        return out if not _want_trace else (out, res)
    total = res.results[0]["out"].astype(np.float64)
    for r in res.results[1:]:
        total = total + r["out"].astype(np.float64)
    out = (total + Wo_b.astype(np.float64)).astype(np.float32)[None]
    return out if not _want_trace else (out, res)
